# revision 1
# baseline (speedup 1.0000x reference)
import sys

sys.path.insert(0, "/opt/trn_rl_repo")

import numpy as np

import concourse.bass as bass
import concourse.tile as tile
from concourse import mybir
from concourse.bass_utils import run_bass_kernel_spmd

# Problem constants (nn_DeltaNet_31877247271467)
B, L, HS = 4, 4096, 1024
NH, DK, DV = 4, 256, 256
CONV, CHUNK, FIRS, FIRL = 4, 32, 5, 64
DECAY = 1.0 - 1.0 / 3000.0
EPS_FLOOR = 0.08 * DECAY
RMS_EPS = 1e-05

FH = 2 * DK  # 512 features per head-half (2 heads of 256)
LB = 512     # L block for device matmul
KO = HS // 128  # 8 contraction tiles


def _build_nc():
    """Per-core SPMD program: qT/kT/vT = W{q,k,v}T_half.T-style projections.

    Inputs (per core): hT (HS, L) = hidden[b].T, w{q,k,v}T (HS, FH) =
    W{q,k,v}[head_half_rows].T.  Outputs: {q,k,v}T (FH, L).
    Contraction over HS on the partition dim, fp32r matmuls (full rate at
    free dim 512), accumulated in PSUM over 8 K-tiles.
    """
    nc = bass.Bass()
    f32 = mybir.dt.float32
    LTOT = L + 3 * FH  # hidden columns then wq|wk|wv weight columns
    X = nc.declare_dram_parameter("X", [HS, LTOT], f32, isOutput=False)
    wouts = {}
    for n in ("q", "k", "v"):
        wouts[n] = nc.declare_dram_parameter(f"{n}T", [FH, L], f32, isOutput=True)

    groups = []
    for lb in range(L // LB):
        for ni, n in enumerate(("q", "k", "v")):
            for m in range(FH // 128):
                groups.append((lb, ni, n, m))
    NG = len(groups)

    with (
        nc.sbuf_tensor([128, KO, LTOT], f32) as xt,
        nc.sbuf_tensor([128, 2, LB], f32) as ob,
        nc.psum_tensor([128, 2, LB], f32) as psum,
        nc.semaphore("dsem") as dsem,
        nc.semaphore("psem") as psem,
        nc.semaphore("vsem") as vsem,
        nc.semaphore("osem") as osem,
        nc.Block() as block,
    ):

        @block.gpsimd
        def _(gps):
            gps.dma_start(
                out=xt[:, :, :], in_=X.rearrange("(ko p) n -> p ko n", p=128)
            ).then_inc(dsem, 16)
            for g, (lb, ni, n, m) in enumerate(groups):
                gps.wait_ge(vsem, g + 1)
                gps.dma_start(
                    out=wouts[n][m * 128 : (m + 1) * 128, lb * LB : (lb + 1) * LB],
                    in_=ob[:, g % 2, :],
                ).then_inc(osem, 16)

        @block.tensor
        def _(pe):
            pe.wait_ge(dsem, 16)
            for g, (lb, ni, n, m) in enumerate(groups):
                wcol = L + ni * FH + m * 128
                if g >= 2:
                    pe.wait_ge(vsem, g - 1)
                for k in range(KO):
                    ins = pe.matmul(
                        psum[:, g % 2, :],
                        xt[:, k, wcol : wcol + 128],
                        xt[:, k, lb * LB : (lb + 1) * LB],
                        start=(k == 0),
                        stop=(k == KO - 1),
                    )
                    if k == KO - 1:
                        ins.then_inc(psem, 1)

        @block.vector
        def _(vec):
            for g in range(NG):
                vec.wait_ge(psem, g + 1)
                if g >= 2:
                    vec.wait_ge(osem, (g - 1) * 16)
                vec.tensor_copy(out=ob[:, g % 2, :], in_=psum[:, g % 2, :]).then_inc(
                    vsem, 1
                )

    return nc


def _dwconv_causal(x, filt):
    # x: (b, l, ch), filt: (ch, K) depthwise causal FIR
    K = filt.shape[-1]
    b, l, ch = x.shape
    y = np.zeros_like(x)
    for k in range(K):
        shift = K - 1 - k  # tap k reads x[t - shift]
        if shift == 0:
            y += filt[:, k] * x
        else:
            y[:, shift:, :] += filt[:, k] * x[:, :-shift, :]
    return y


def _silu(x):
    return x / (1.0 + np.exp(-x)) * np.ones((), np.float32)


def _sigmoid(x):
    return 1.0 / (1.0 + np.exp(-x))


def _gelu_tanh(x):
    c = np.float32(np.sqrt(2.0 / np.pi))
    return 0.5 * x * (1.0 + np.tanh(c * (x + 0.044715 * x * x * x)))


def _l2norm(x):
    return x / np.sqrt(np.sum(x * x, -1, keepdims=True) + 1e-6)


def _delta_rule_chunkwise(q, k, v, beta, chunk=CHUNK):
    b, h, Lq, dk = q.shape
    dv = v.shape[-1]
    n = Lq // chunk
    q = _l2norm(q).astype(np.float32)
    k = _l2norm(k).astype(np.float32)
    v = (v * beta[..., None]).astype(np.float32)
    kb = (k * beta[..., None]).astype(np.float32)
    r = lambda x: x.reshape(b, h, n, chunk, dv if x.shape[-1] == dv else dk)
    q, k, v, kb = r(q), r(k), r(v), r(kb)
    strict_low = np.tril(np.ones((chunk, chunk), bool), -1)
    A = np.where(strict_low, -np.einsum("bhnid,bhnjd->bhnij", kb, k), 0.0).astype(
        np.float32
    )
    eye = np.eye(chunk, dtype=np.float64)
    T = np.linalg.inv(eye - A.astype(np.float64)).astype(np.float32)
    u = T @ v
    w = T @ kb
    low = np.tril(np.ones((chunk, chunk), bool))
    S = np.zeros((b, h, dk, dv), np.float32)
    o = np.empty((b, h, n, chunk, dv), np.float32)
    for i in range(n):
        qi, ki, ui, wi = q[:, :, i], k[:, :, i], u[:, :, i], w[:, :, i]
        attn = np.where(low, np.einsum("bhid,bhjd->bhij", qi, ki), 0.0).astype(
            np.float32
        )
        u_i = ui - wi @ S
        o[:, :, i] = qi @ S + attn @ u_i
        S = S + np.einsum("bhcd,bhce->bhde", ki, u_i)
    return o.reshape(b, h, Lq, dv)


def _stats(x):
    mean = np.mean(x, -1, keepdims=True)
    var = np.var(x, -1, keepdims=True)
    am = np.mean(np.abs(x), -1, keepdims=True)
    l2 = np.sqrt(np.sum(x * x, -1, keepdims=True))
    return np.concatenate([mean, var, am, l2], -1).astype(np.float32)


def kernel(
    hidden_states,
    Wq,
    Wk,
    Wv,
    Wb,
    conv_q_w,
    conv_k_w,
    conv_v_w,
    fir_short_filt,
    fir_long_filt,
    gate_W1,
    gate_b1,
    gate_W2,
    gate_b2,
    gate_copy_bias,
    gate_log_temp,
    o_norm_w,
    Wo,
):
    hidden_states = np.asarray(hidden_states, np.float32)
    b, l, _ = hidden_states.shape

    # ---- device: q/k/v projections, sharded over (batch, head-half) on 8 cores
    nc = _build_nc()
    hT = np.ascontiguousarray(hidden_states.transpose(0, 2, 1))  # (B, HS, L)
    in_maps = []
    for c in range(8):
        bb, hg = c // 2, c % 2
        rows = slice(hg * FH, (hg + 1) * FH)
        X = np.concatenate(
            [
                hT[bb],
                np.asarray(Wq, np.float32)[rows].T,
                np.asarray(Wk, np.float32)[rows].T,
                np.asarray(Wv, np.float32)[rows].T,
            ],
            axis=1,
        )
        in_maps.append({"X": np.ascontiguousarray(X)})
    res = run_bass_kernel_spmd(nc, in_maps, list(range(8))).results

    def gather(name):
        out = np.empty((B, l, NH * DK), np.float32)
        for c in range(8):
            bb, hg = c // 2, c % 2
            out[bb, :, hg * FH : (hg + 1) * FH] = np.asarray(res[c][name]).T
        return out

    q_pre, k_pre, v_pre = gather("qT"), gather("kT"), gather("vT")

    # ---- host: the rest of the module in fp32 numpy
    q = _silu(_dwconv_causal(q_pre, np.asarray(conv_q_w, np.float32)))
    k = _silu(_dwconv_causal(k_pre, np.asarray(conv_k_w, np.float32)))
    v = _silu(_dwconv_causal(v_pre, np.asarray(conv_v_w, np.float32)))
    beta = _sigmoid(hidden_states @ np.asarray(Wb, np.float32).T)  # (b,l,h)

    qh = q.reshape(b, l, NH, DK).transpose(0, 2, 1, 3)
    kh = k.reshape(b, l, NH, DK).transpose(0, 2, 1, 3)
    vh = v.reshape(b, l, NH, DV).transpose(0, 2, 1, 3)
    o_d = _delta_rule_chunkwise(qh, kh, vh, beta.transpose(0, 2, 1))
    o_d = o_d.transpose(0, 2, 1, 3)  # (b,l,h,dv)

    v_direct = v.reshape(b, l, NH, DV)
    vc = v_direct.reshape(b, l, NH * DV)
    fir_s = _dwconv_causal(
        vc, np.asarray(fir_short_filt, np.float32).reshape(NH * DV, FIRS)
    ).reshape(b, l, NH, DV)
    fir_l = _dwconv_causal(
        vc, np.asarray(fir_long_filt, np.float32).reshape(NH * DV, FIRL)
    ).reshape(b, l, NH, DV)

    stats = np.concatenate(
        [_stats(fir_s), _stats(fir_l), _stats(o_d), _stats(v_direct)], -1
    )
    gin = np.concatenate(
        [np.broadcast_to(hidden_states[:, :, None, :], (b, l, NH, HS)), stats], -1
    ).astype(np.float32)
    h1 = _gelu_tanh(gin @ np.asarray(gate_W1, np.float32).T + np.asarray(gate_b1, np.float32))
    logits = h1 @ np.asarray(gate_W2, np.float32).T + np.asarray(gate_b2, np.float32)
    bias_val = np.asarray(gate_copy_bias, np.float32) * DECAY
    logits = logits + bias_val[None, None, :, None] * np.array(
        [0.0, 0.0, 0.0, 1.0], np.float32
    )
    temp = np.exp(np.asarray(gate_log_temp, np.float32))
    z = logits / temp[None, None, :, None]
    z = z - z.max(-1, keepdims=True)
    ez = np.exp(z)
    wgt = ez / ez.sum(-1, keepdims=True)
    wgt = wgt * (1.0 - 4.0 * EPS_FLOOR) + EPS_FLOOR
    o = (
        wgt[..., 0:1] * fir_s
        + wgt[..., 1:2] * fir_l
        + wgt[..., 2:3] * o_d
        + wgt[..., 3:4] * v_direct
    )
    o = (
        o
        / np.sqrt(np.mean(o * o, -1, keepdims=True) + RMS_EPS)
        * np.asarray(o_norm_w, np.float32)
    )
    return (o.reshape(b, l, NH * DV) @ np.asarray(Wo, np.float32).T).astype(np.float32)



# revision 21
# speedup vs baseline: 21.8649x; 21.8649x over previous
"""Fused DeltaNet forward on 8 NeuronCores.

Sharding: core c handles batch b=c//2 and head-pair hg=c%2 (heads 2hg, 2hg+1).
The entire module (projections, causal conv+silu, l2norm, chunkwise delta rule
with chunk=128 via Neumann-doubling triangular inverse, FIR branches, stats,
gate MLP, softmax gate, rmsnorm, partial output projection) runs on-device in
one Bass/Tile program. The host only packs inputs (bf16) and sums the two
head-pair partial outputs per batch.
"""

import sys

sys.path.insert(0, "/opt/trn_rl_repo")

import numpy as np
import ml_dtypes

import concourse.bass as bass
import concourse.bacc as bacc
import concourse.tile as tile
from concourse import mybir
from concourse.bass import ds
from concourse.masks import make_identity

BF = ml_dtypes.bfloat16
F32 = mybir.dt.float32
BF16 = mybir.dt.bfloat16

# Problem constants
B, L, HS = 4, 4096, 1024
NH, DK, DV = 4, 256, 256
CONV, FIRS, FIRL = 4, 5, 64
CHUNK = 128
NCH = L // CHUNK          # 32 chunks
TTS = L // 512            # 8 t-tiles of 512
Q0 = 3                    # zero-pad columns at the head of q/k tiles
DECAY = 1.0 - 1.0 / 3000.0
EPS_FLOOR = 0.08 * DECAY
RMS_EPS = 1e-05
AFT = mybir.ActivationFunctionType
ALU = mybir.AluOpType

# --- bf16 blob layout (per core) ---
SEC_BF = {}
_o = 0
for name, sz in [
    ("hT", HS * L),          # hidden[b].T, (1024, 4096)
    ("wq", HS * 512),        # Wq[rows].T  (1024, 512)
    ("wk", HS * 512),
    ("wv", HS * 512),
    ("wb", HS * 2),          # Wb[h0:h0+2].T (1024, 2)
    ("w1h", HS * 1024),      # W1h.T (1024, 1024)
    ("w1s", 64 * 1024),      # W1s.T duplicated rows at 0-15 and 32-47 (64, 1024)
    ("w2", HS * 2 * 4),      # W2.T scaled per head (1024, 2, 4)
    ("wo", 512 * 1024),      # Wo'[:, rows].T (512, 1024)
]:
    SEC_BF[name] = (_o, sz)
    _o += sz
NBF = _o

SEC_F = {}
_o = 0
for name, sz in [
    ("cw", 128 * 4 * 3 * 4),    # conv taps   [p, ftile, proj, tap]
    ("fir", 128 * 4 * (FIRS + FIRL)),  # fir taps [p, ftile, 0:5 fs | 5:69 fl]
    ("msk", 128 * 256),         # [:, :128] strict-lower(-1); [:, 128:] upper-incl(+1)
    ("b1", 128 * 8),            # gate_b1 [p, f1tile]
    ("b2", 4 * 2),              # exp bias per head [j, h]
]:
    SEC_F[name] = (_o, sz)
    _o += sz
NF = _o


def _bfs(blob, name, shape_str, **axes):
    o, sz = SEC_BF[name]
    return blob[ds(o, sz)].rearrange(shape_str, **axes)


def _fs(blob, name, shape_str, **axes):
    o, sz = SEC_F[name]
    return blob[ds(o, sz)].rearrange(shape_str, **axes)


def _build_nc():
    import contextlib

    nc = bacc.Bacc()
    bfb = nc.declare_dram_parameter("bfb", [NBF], BF16, isOutput=False)
    f32b = nc.declare_dram_parameter("f32b", [NF], F32, isOutput=False)
    out_d = nc.declare_dram_parameter("out", [HS, L], BF16, isOutput=True)

    hT_ap = _bfs(bfb, "hT", "(kt p t) -> p kt t", p=128, t=L)
    out_ap = out_d.rearrange("(ot p) t -> p ot t", p=128)
    o_wq, _ = SEC_BF["wq"]
    wqkv_ap = bfb[ds(o_wq, 3 * HS * 512)].rearrange(
        "(j kt p n) -> p kt j n", j=3, kt=8, p=128
    )

    with tile.TileContext(nc, pool_alloc_mode="queue") as tc:
        root = contextlib.ExitStack()
        with root:
            consts = root.enter_context(tc.tile_pool(name="consts", bufs=1))
            id_f32 = consts.tile([128, 128], F32, tag="idf")
            make_identity(nc, id_f32)
            ones_f = consts.tile([128, 128], F32, tag="onesf")
            nc.gpsimd.memset(ones_f, 1.0)
            ones_b = consts.tile([128, 1], BF16, tag="onesb")
            nc.gpsimd.memset(ones_b, 1.0)
            msk = consts.tile([128, 256], F32, tag="msk")
            nc.sync.dma_start(out=msk, in_=_fs(f32b, "msk", "(p n) -> p n", p=128))
            cw = consts.tile([128, 4, 3, 4], F32, tag="cw")
            nc.sync.dma_start(
                out=cw, in_=_fs(f32b, "cw", "(p a b c) -> p a b c", p=128, a=4, b=3)
            )
            fw = consts.tile([128, 4, FIRS + FIRL], F32, tag="fw")
            nc.sync.dma_start(
                out=fw, in_=_fs(f32b, "fir", "(p a k) -> p a k", p=128, a=4)
            )
            b1t = consts.tile([128, 8], F32, tag="b1")
            nc.sync.dma_start(out=b1t, in_=_fs(f32b, "b1", "(p a) -> p a", p=128))
            b2t = consts.tile([4, 2], F32, tag="b2")
            nc.sync.dma_start(out=b2t, in_=_fs(f32b, "b2", "(p a) -> p a", p=4))
            epsc = consts.tile([128, 2], F32, tag="epsc")
            nc.gpsimd.memset(epsc[:, 0:1], 1e-6)
            nc.gpsimd.memset(epsc[:, 1:2], RMS_EPS)
            bT = consts.tile([128, NCH, 2], F32, tag="bT")
            S = consts.tile([128, 2, 2, 256], F32, tag="S")
            nc.gpsimd.memset(S, 0.0)
            stats = consts.tile([64, TTS, 512], BF16, tag="stats")
            w1s = consts.tile([64, 1024], BF16, tag="w1s")
            nc.sync.dma_start(out=w1s, in_=_bfs(bfb, "w1s", "(p n) -> p n", p=64))

            vpool = root.enter_context(tc.tile_pool(name="vpool", bufs=1))
            vt = vpool.tile([128, 4, 64 + L], BF16, tag="v")
            nc.vector.memset(vt[:, :, 0:64], 0.0)

            qkpool = root.enter_context(tc.tile_pool(name="qkpool", bufs=1))
            qt = qkpool.tile([128, 4, Q0 + L], BF16, tag="q")
            kt = qkpool.tile([128, 4, Q0 + L], BF16, tag="k")
            nc.vector.memset(qt[:, :, 0:Q0], 0.0)
            nc.vector.memset(kt[:, :, 0:Q0], 0.0)

            # ---------------- Stage A: projections + conv(inplace) + silu + l2norm
            with contextlib.ExitStack() as sa:
                pa = sa.enter_context(tc.tile_pool(name="pa", bufs=1))
                dbl = sa.enter_context(tc.tile_pool(name="dbl", bufs=2))
                psA = sa.enter_context(tc.tile_pool(name="psA", bufs=2, space="PSUM"))
                psR = sa.enter_context(tc.tile_pool(name="psR", bufs=2, space="PSUM"))
                psB = sa.enter_context(tc.tile_pool(name="psB", bufs=2, space="PSUM"))

                wsl = pa.tile([128, 8, 3, 512], BF16, tag="wqkv")
                for _j, _wn in enumerate(("wq", "wk", "wv")):
                    nc.sync.dma_start(
                        out=wsl[:, :, _j, :],
                        in_=_bfs(bfb, _wn, "(kt p n) -> p kt n", p=128, n=512),
                    )
                wbsl = pa.tile([128, 8, 2], BF16, tag="wb")
                nc.sync.dma_start(
                    out=wbsl, in_=_bfs(bfb, "wb", "(kt p n) -> p kt n", p=128, n=2)
                )
                dests = [(qt, Q0), (kt, Q0), (vt, 64)]
                for tt in range(TTS):
                    hsl = dbl.tile([128, 8, 512], BF16, tag="hsl")
                    nc.gpsimd.dma_start(out=hsl, in_=hT_ap[:, :, ds(tt * 512, 512)])
                    for pj in range(3):
                        dest, dofs = dests[pj]
                        for ft in range(4):
                            ps = psA.tile([128, 512], F32, tag="psA")
                            for g in range(8):
                                nc.tensor.matmul(
                                    ps,
                                    wsl[:, g, pj, ds(ft * 128, 128)],
                                    hsl[:, g, :],
                                    start=(g == 0),
                                    stop=(g == 7),
                                )
                            nc.vector.tensor_copy(
                                out=dest[:, ft, ds(dofs + tt * 512, 512)], in_=ps
                            )
                    # beta
                    psb = psR.tile([2, 512], F32, tag="psb")
                    for g in range(8):
                        nc.tensor.matmul(
                            psb, wbsl[:, g, :], hsl[:, g, :],
                            start=(g == 0), stop=(g == 7),
                        )
                    brow = dbl.tile([2, 512], F32, tag="brow")
                    nc.scalar.activation(out=brow, in_=psb, func=AFT.Sigmoid)
                    for g2 in range(4):
                        pst = psR.tile([128, 2], F32, tag="psb")
                        nc.tensor.transpose(
                            pst, brow[0:2, ds(g2 * 128, 128)], id_f32[0:2, 0:2]
                        )
                        nc.vector.tensor_copy(out=bT[:, tt * 4 + g2, :], in_=pst)

                # conv in place (right-to-left t-tiles), then silu in place
                for pj in range(3):
                    dest, dofs = dests[pj]
                    for ft in range(4):
                        for tt in reversed(range(TTS)):
                            acc = dbl.tile([128, 512], F32, tag="cacc")
                            nc.vector.tensor_scalar_mul(
                                acc,
                                dest[:, ft, ds(dofs + tt * 512, 512)],
                                cw[:, ft, pj, 3:4],
                            )
                            for k in range(3):
                                nc.vector.scalar_tensor_tensor(
                                    out=acc,
                                    in0=dest[:, ft, ds(dofs - 3 + k + tt * 512, 512)],
                                    scalar=cw[:, ft, pj, ds(k, 1)],
                                    in1=acc,
                                    op0=ALU.mult,
                                    op1=ALU.add,
                                )
                            nc.scalar.activation(
                                out=dest[:, ft, ds(dofs + tt * 512, 512)],
                                in_=acc,
                                func=AFT.Silu,
                            )

                # l2norm q, k per head
                for x, dofs in ((qt, Q0), (kt, Q0)):
                    for h in range(2):
                        for tt in range(TTS):
                            xsq = dbl.tile([128, 2, 512], BF16, tag="xsq")
                            for f2 in range(2):
                                nc.scalar.square(
                                    xsq[:, f2, :],
                                    x[:, 2 * h + f2, ds(dofs + tt * 512, 512)],
                                )
                            ps = psR.tile([1, 512], F32, tag="psl2")
                            nc.tensor.matmul(
                                ps, ones_b, xsq[:, 0, :], start=True, stop=False
                            )
                            nc.tensor.matmul(
                                ps, ones_b, xsq[:, 1, :], start=False, stop=True
                            )
                            srow = dbl.tile([1, 512], F32, tag="srow")
                            nc.scalar.activation(
                                out=srow, in_=ps, func=AFT.Sqrt, bias=epsc[0:1, 0:1]
                            )
                            nc.vector.reciprocal(srow, srow)
                            rb = psB.tile([128, 512], F32, tag="psBb")
                            nc.tensor.matmul(
                                rb, ones_f[0:1, :], srow, start=True, stop=True
                            )
                            for f2 in range(2):
                                sl = x[:, 2 * h + f2, ds(dofs + tt * 512, 512)]
                                nc.vector.tensor_mul(sl, sl, rb)

            # ---------------- Stage B: delta rule, chunk=128
            odpool = root.enter_context(tc.tile_pool(name="odpool", bufs=1))
            od = odpool.tile([128, 4, L], BF16, tag="od")

            with contextlib.ExitStack() as sb:
                dl = sb.enter_context(tc.tile_pool(name="dl", bufs=2))
                ps1 = sb.enter_context(tc.tile_pool(name="ps1", bufs=3, space="PSUM"))
                ps2 = sb.enter_context(tc.tile_pool(name="ps2", bufs=3, space="PSUM"))
                psD = sb.enter_context(tc.tile_pool(name="psD", bufs=1, space="PSUM"))

                for ci in range(NCH):
                    csl = ds(Q0 + ci * CHUNK, CHUNK)
                    vsl = ds(64 + ci * CHUNK, CHUNK)
                    osl = ds(ci * CHUNK, CHUNK)
                    for h in range(2):
                        bcol = bT[:, ci, ds(h, 1)]
                        qf = dl.tile([128, 256], F32, tag="qf")
                        nc.vector.tensor_copy(out=qf, in_=qt[:, ds(2 * h, 2), csl])
                        kf = dl.tile([128, 256], F32, tag="kf")
                        nc.vector.tensor_copy(out=kf, in_=kt[:, ds(2 * h, 2), csl])
                        vf = dl.tile([128, 256], F32, tag="vf")
                        nc.vector.tensor_copy(out=vf, in_=vt[:, ds(2 * h, 2), vsl])
                        kcdk = dl.tile([128, 256], F32, tag="kcdk")
                        vcdv = dl.tile([128, 256], F32, tag="vcdv")
                        for f2 in range(2):
                            pt = ps1.tile([128, 128], F32, tag="ps1")
                            nc.tensor.transpose(pt, kf[:, ds(f2 * 128, 128)], id_f32)
                            nc.vector.tensor_copy(out=kcdk[:, ds(f2 * 128, 128)], in_=pt)
                            pt2 = ps1.tile([128, 128], F32, tag="ps1")
                            nc.tensor.transpose(pt2, vf[:, ds(f2 * 128, 128)], id_f32)
                            nc.vector.tensor_copy(out=vcdv[:, ds(f2 * 128, 128)], in_=pt2)
                        nc.vector.tensor_scalar_mul(vcdv, vcdv, bcol)
                        kb = dl.tile([128, 256], F32, tag="kb")
                        nc.vector.tensor_scalar_mul(kb, kcdk, bcol)
                        # A
                        aps = ps1.tile([128, 128], F32, tag="ps1")
                        nc.tensor.matmul(
                            aps, kf[:, 0:128], kf[:, 0:128], start=True, stop=False
                        )
                        nc.tensor.matmul(
                            aps, kf[:, 128:256], kf[:, 128:256], start=False, stop=True
                        )
                        A = dl.tile([128, 128], F32, tag="A")
                        nc.vector.scalar_tensor_tensor(
                            out=A, in0=aps, scalar=bcol, in1=msk[:, 0:128],
                            op0=ALU.mult, op1=ALU.mult,
                        )
                        atp = ps1.tile([128, 128], F32, tag="ps1")
                        nc.tensor.transpose(atp, A, id_f32)
                        PT = dl.tile([128, 128], F32, tag="PT")
                        nc.vector.tensor_copy(out=PT, in_=atp)
                        X = dl.tile([128, 128], F32, tag="X")
                        nc.vector.tensor_add(out=X, in0=atp, in1=id_f32)
                        P = A
                        for lvl in range(6):
                            pps = ps1.tile([128, 128], F32, tag="ps1")
                            nc.tensor.matmul(pps, PT, P, start=True, stop=True)
                            if lvl < 5:
                                Pn = dl.tile([128, 128], F32, tag="Pn")
                                nc.vector.tensor_copy(out=Pn, in_=pps)
                            IP = dl.tile([128, 128], F32, tag="IP")
                            nc.vector.tensor_add(out=IP, in0=pps, in1=id_f32)
                            xps = ps1.tile([128, 128], F32, tag="ps1")
                            nc.tensor.matmul(xps, IP, X, start=True, stop=True)
                            Xn = dl.tile([128, 128], F32, tag="X")
                            nc.vector.tensor_copy(out=Xn, in_=xps)
                            X = Xn
                            if lvl < 5:
                                ptp = ps1.tile([128, 128], F32, tag="ps1")
                                nc.tensor.transpose(ptp, Pn, id_f32)
                                PTn = dl.tile([128, 128], F32, tag="PT")
                                nc.vector.tensor_copy(out=PTn, in_=ptp)
                                P, PT = Pn, PTn
                        ups = ps2.tile([128, 256], F32, tag="ps2")
                        nc.tensor.matmul(ups, X, vcdv, start=True, stop=True)
                        u = dl.tile([128, 256], F32, tag="u")
                        nc.vector.tensor_copy(out=u, in_=ups)
                        wps = ps2.tile([128, 256], F32, tag="ps2")
                        nc.tensor.matmul(wps, X, kb, start=True, stop=True)
                        w = dl.tile([128, 256], F32, tag="w")
                        nc.vector.tensor_copy(out=w, in_=wps)
                        wT = dl.tile([128, 2, 128], F32, tag="wT")
                        for f2 in range(2):
                            tp = ps1.tile([128, 128], F32, tag="ps1")
                            nc.tensor.transpose(tp, w[:, ds(f2 * 128, 128)], id_f32)
                            nc.vector.tensor_copy(out=wT[:, f2, :], in_=tp)
                        ats = ps1.tile([128, 128], F32, tag="ps1")
                        nc.tensor.matmul(
                            ats, kf[:, 0:128], qf[:, 0:128], start=True, stop=False
                        )
                        nc.tensor.matmul(
                            ats, kf[:, 128:256], qf[:, 128:256], start=False, stop=True
                        )
                        at = dl.tile([128, 128], F32, tag="at")
                        nc.vector.tensor_mul(at, ats, msk[:, 128:256])
                        # sequential
                        wsp = ps2.tile([128, 256], F32, tag="ps2")
                        nc.tensor.matmul(
                            wsp, wT[:, 0, :], S[:, h, 0, :], start=True, stop=False
                        )
                        nc.tensor.matmul(
                            wsp, wT[:, 1, :], S[:, h, 1, :], start=False, stop=True
                        )
                        nc.vector.tensor_sub(u, u, wsp)
                        osp = ps2.tile([128, 256], F32, tag="ps2")
                        nc.tensor.matmul(
                            osp, qf[:, 0:128], S[:, h, 0, :], start=True, stop=False
                        )
                        nc.tensor.matmul(
                            osp, qf[:, 128:256], S[:, h, 1, :], start=False, stop=False
                        )
                        nc.tensor.matmul(osp, at, u, start=False, stop=True)
                        dsp = psD.tile([128, 2, 256], F32, tag="psD")
                        nc.tensor.matmul(
                            dsp[:, 0, :], kcdk[:, 0:128], u, start=True, stop=True
                        )
                        nc.tensor.matmul(
                            dsp[:, 1, :], kcdk[:, 128:256], u, start=True, stop=True
                        )
                        nc.vector.tensor_add(S[:, h, 0, :], S[:, h, 0, :], dsp[:, 0, :])
                        nc.vector.tensor_add(S[:, h, 1, :], S[:, h, 1, :], dsp[:, 1, :])
                        ob = dl.tile([128, 256], F32, tag="ob")
                        nc.vector.tensor_copy(out=ob, in_=osp)
                        for f2 in range(2):
                            otp = ps1.tile([128, 128], F32, tag="ps1")
                            nc.tensor.transpose(otp, ob[:, ds(f2 * 128, 128)], id_f32)
                            nc.vector.tensor_copy(out=od[:, 2 * h + f2, osl], in_=otp)

            # helper: recompute fir_s slice (5-tap) into dst
            def emit_fir_s(dst, ft, t0, n):
                nc.vector.tensor_scalar_mul(
                    dst, vt[:, ft, ds(64 + t0, n)], fw[:, ft, 4:5]
                )
                for k in range(4):
                    nc.vector.scalar_tensor_tensor(
                        out=dst,
                        in0=vt[:, ft, ds(60 + k + t0, n)],
                        scalar=fw[:, ft, ds(k, 1)],
                        in1=dst,
                        op0=ALU.mult,
                        op1=ALU.add,
                    )

            # ---------------- Stage C: FIR long (into qt) + stats
            flt = qt  # reuse q's SBUF; fir_l values live at [:, ft, Q0 + t]

            with contextlib.ExitStack() as sc:
                cs = sc.enter_context(tc.tile_pool(name="cs", bufs=2))
                cs1 = sc.enter_context(tc.tile_pool(name="cs1", bufs=1))
                psS = sc.enter_context(tc.tile_pool(name="psS", bufs=3, space="PSUM"))

                for ft in range(4):
                    facc = cs1.tile([128, L], F32, tag="facc")
                    nc.vector.tensor_scalar_mul(
                        facc, vt[:, ft, ds(64, L)], fw[:, ft, ds(5 + 63, 1)]
                    )
                    for k in range(63):
                        nc.vector.scalar_tensor_tensor(
                            out=facc,
                            in0=vt[:, ft, ds(1 + k, L)],
                            scalar=fw[:, ft, ds(5 + k, 1)],
                            in1=facc,
                            op0=ALU.mult,
                            op1=ALU.add,
                        )
                    nc.vector.tensor_copy(out=flt[:, ft, ds(Q0, L)], in_=facc)

                for h in range(2):
                    for tt in range(TTS):
                        for src in range(4):
                            xa = cs.tile([128, 2, 512], BF16, tag="xa")
                            xq = cs.tile([128, 2, 512], BF16, tag="xq")
                            if src == 0:
                                fsr = cs1.tile([128, 2, 512], F32, tag="fsr")
                                for f2 in range(2):
                                    emit_fir_s(fsr[:, f2, :], 2 * h + f2, tt * 512, 512)

                            def srcsl(f2):
                                if src == 0:
                                    return fsr[:, f2, :]
                                if src == 1:
                                    return flt[:, 2 * h + f2, ds(Q0 + tt * 512, 512)]
                                if src == 2:
                                    return od[:, 2 * h + f2, ds(tt * 512, 512)]
                                return vt[:, 2 * h + f2, ds(64 + tt * 512, 512)]

                            for f2 in range(2):
                                nc.scalar.activation(
                                    out=xa[:, f2, :], in_=srcsl(f2), func=AFT.Abs
                                )
                                nc.scalar.square(xq[:, f2, :], srcsl(f2))
                            pS = psS.tile([1, 512], F32, tag="psS")
                            pQ = psS.tile([1, 512], F32, tag="psS")
                            pA = psS.tile([1, 512], F32, tag="psS")
                            for f2 in range(2):
                                st, sp = (f2 == 0), (f2 == 1)
                                if src == 0:
                                    nc.tensor.matmul(
                                        pS, ones_f[:, 0:1], srcsl(f2), start=st, stop=sp
                                    )
                                else:
                                    nc.tensor.matmul(
                                        pS, ones_b, srcsl(f2), start=st, stop=sp
                                    )
                                nc.tensor.matmul(
                                    pQ, ones_b, xq[:, f2, :], start=st, stop=sp
                                )
                                nc.tensor.matmul(
                                    pA, ones_b, xa[:, f2, :], start=st, stop=sp
                                )
                            b4 = src * 4
                            r0 = cs.tile([1, 512], BF16, tag="r0")
                            r1 = cs.tile([1, 512], BF16, tag="r1")
                            r2 = cs.tile([1, 512], BF16, tag="r2")
                            r3 = cs.tile([1, 512], BF16, tag="r3")
                            nc.vector.tensor_copy(out=r0, in_=pS)
                            tmp = cs.tile([1, 512], F32, tag="tmpr")
                            nc.scalar.square(tmp, pS)
                            nc.vector.tensor_scalar_mul(r1, pQ, 1.0 / 256.0)
                            nc.vector.scalar_tensor_tensor(
                                out=r1, in0=tmp, scalar=-1.0 / 65536.0,
                                in1=r1, op0=ALU.mult, op1=ALU.add,
                            )
                            nc.scalar.activation(out=r3, in_=pQ, func=AFT.Sqrt)
                            nc.vector.tensor_copy(out=r2, in_=pA)
                            for ri, rr in enumerate((r0, r1, r2, r3)):
                                nc.gpsimd.dma_start(
                                    out=stats[ds(h * 32 + b4 + ri, 1), tt, :],
                                    in_=rr,
                                )

            # ---------------- Stage D: gate MLP + combine + rms + Wo partial
            with contextlib.ExitStack() as sd:
                gs = sd.enter_context(tc.tile_pool(name="gs", bufs=1))
                gh = sd.enter_context(tc.tile_pool(name="gh", bufs=2))
                gr = sd.enter_context(tc.tile_pool(name="gr", bufs=2))
                psG = sd.enter_context(tc.tile_pool(name="psG", bufs=3, space="PSUM"))
                psL = sd.enter_context(tc.tile_pool(name="psL", bufs=3, space="PSUM"))
                psW = sd.enter_context(tc.tile_pool(name="psW", bufs=2, space="PSUM"))

                # weights live in kt's (now dead) SBUF
                ktf = kt.rearrange("p a t -> p (a t)")
                w1h = ktf[:, ds(0, 8192)].rearrange("p (g n) -> p g n", n=1024)
                nc.sync.dma_start(
                    out=w1h, in_=_bfs(bfb, "w1h", "(kt p n) -> p kt n", p=128, n=1024)
                )
                wo = ktf[:, ds(8192, 4096)].rearrange("p (g n) -> p g n", n=1024)
                nc.sync.dma_start(
                    out=wo, in_=_bfs(bfb, "wo", "(kt p n) -> p kt n", p=128, n=1024)
                )
                w2 = ktf[:, ds(12288, 64)].rearrange("p (g h j) -> p g h j", h=2, j=4)
                nc.sync.dma_start(
                    out=w2,
                    in_=_bfs(bfb, "w2", "(kt p h j) -> p kt h j", p=128, h=2, j=4),
                )
                on = ktf[:, ds(12352, 2048)].rearrange("p (a n) -> p a n", n=512)

                for tt in range(TTS):
                    tsl = ds(tt * 512, 512)
                    hsl2 = gh.tile([128, 8, 512], BF16, tag="hsl2")
                    nc.gpsimd.dma_start(out=hsl2, in_=hT_ap[:, :, tsl])
                    for h in range(2):
                        h1 = gs.tile([128, 8, 512], BF16, tag="h1")
                        for f1 in range(8):
                            hp = psG.tile([128, 512], F32, tag="psG")
                            for g in range(8):
                                nc.tensor.matmul(
                                    hp,
                                    w1h[:, g, ds(f1 * 128, 128)],
                                    hsl2[:, g, :],
                                    start=(g == 0),
                                    stop=False,
                                )
                            nc.tensor.matmul(
                                hp,
                                w1s[ds(h * 32, 16), ds(f1 * 128, 128)],
                                stats[ds(h * 32, 16), tt, :],
                                start=False,
                                stop=True,
                            )
                            nc.scalar.activation(
                                out=h1[:, f1, :], in_=hp,
                                func=AFT.Gelu_apprx_tanh, bias=b1t[:, ds(f1, 1)],
                            )
                        lg = psL.tile([4, 512], F32, tag="psL")
                        for g in range(8):
                            nc.tensor.matmul(
                                lg, w2[:, g, h, :], h1[:, g, :],
                                start=(g == 0), stop=(g == 7),
                            )
                        ez = gr.tile([4, 512], F32, tag="r4")
                        nc.scalar.activation(
                            out=ez, in_=lg, func=AFT.Exp, bias=b2t[:, ds(h, 1)]
                        )
                        sps = psL.tile([4, 512], F32, tag="psL")
                        nc.tensor.matmul(
                            sps[0:1, :], ones_f[0:4, 0:1], ez, start=True, stop=True
                        )
                        srec = gr.tile([1, 512], F32, tag="r1")
                        nc.vector.reciprocal(srec, sps[0:1, :])
                        nc.vector.tensor_scalar_mul(srec, srec, 1.0 - 4.0 * EPS_FLOOR)
                        srb = psL.tile([4, 512], F32, tag="psL")
                        nc.tensor.matmul(
                            srb, ones_f[0:1, 0:4], srec, start=True, stop=True
                        )
                        wgt = gr.tile([4, 512], F32, tag="r4")
                        nc.vector.tensor_mul(wgt, ez, srb)
                        nc.vector.tensor_scalar_add(wgt, wgt, EPS_FLOOR)
                        o_acc = gs.tile([128, 2, 512], F32, tag="oacc")
                        for j in range(4):
                            rsp = psL.tile([4, 512], F32, tag="psL")
                            nc.tensor.matmul(
                                rsp[0:1, :], id_f32[0:4, ds(j, 1)], wgt,
                                start=True, stop=True,
                            )
                            rrow = gr.tile([1, 512], F32, tag="r1")
                            nc.vector.tensor_copy(out=rrow, in_=rsp[0:1, :])
                            wb_ = psG.tile([128, 512], F32, tag="psG")
                            nc.tensor.matmul(
                                wb_, ones_f[0:1, :], rrow, start=True, stop=True
                            )
                            for f2 in range(2):
                                if j == 0:
                                    scr = gs.tile([128, 512], F32, tag="scr")
                                    emit_fir_s(scr, 2 * h + f2, tt * 512, 512)
                                    nc.vector.tensor_mul(o_acc[:, f2, :], scr, wb_)
                                else:
                                    if j == 1:
                                        sl = flt[:, 2 * h + f2, ds(Q0 + tt * 512, 512)]
                                    elif j == 2:
                                        sl = od[:, 2 * h + f2, tsl]
                                    else:
                                        sl = vt[:, 2 * h + f2, ds(64 + tt * 512, 512)]
                                    scr = gs.tile([128, 512], F32, tag="scr")
                                    nc.vector.tensor_mul(scr, sl, wb_)
                                    nc.vector.tensor_add(
                                        o_acc[:, f2, :], o_acc[:, f2, :], scr
                                    )
                        # rms over this head's 256 features
                        rps = psL.tile([4, 512], F32, tag="psL")
                        for f2 in range(2):
                            scr = gs.tile([128, 512], F32, tag="scr")
                            nc.scalar.square(scr, o_acc[:, f2, :])
                            nc.tensor.matmul(
                                rps[0:1, :], ones_f[:, 0:1], scr,
                                start=(f2 == 0), stop=(f2 == 1),
                            )
                        rr2 = gr.tile([1, 512], F32, tag="r1")
                        nc.scalar.activation(
                            out=rr2, in_=rps[0:1, :], func=AFT.Sqrt,
                            bias=epsc[0:1, 1:2], scale=1.0 / 256.0,
                        )
                        nc.vector.reciprocal(rr2, rr2)
                        rb2 = psG.tile([128, 512], F32, tag="psG")
                        nc.tensor.matmul(rb2, ones_f[0:1, :], rr2, start=True, stop=True)
                        for f2 in range(2):
                            nc.vector.tensor_mul(
                                on[:, 2 * h + f2, :], o_acc[:, f2, :], rb2
                            )
                    # Wo partial: two halves of 4 of-tiles each
                    for half in range(2):
                        obuf = gs.tile([128, 4, 512], BF16, tag="obuf")
                        for o4 in range(4):
                            of = half * 4 + o4
                            wp = psW.tile([128, 512], F32, tag="psW")
                            for g in range(4):
                                nc.tensor.matmul(
                                    wp,
                                    wo[:, g, ds(of * 128, 128)],
                                    on[:, g, :],
                                    start=(g == 0),
                                    stop=(g == 3),
                                )
                            nc.vector.tensor_copy(out=obuf[:, o4, :], in_=wp)
                        nc.gpsimd.dma_start(
                            out=out_ap[:, ds(half * 4, 4), tsl], in_=obuf
                        )
    nc.compile()
    return nc


# ---------------- host side ----------------

_NC = None
_RUN = None


def _make_runner():
    global _RUN
    import jax
    from jax.sharding import Mesh, PartitionSpec
    from jax.experimental.shard_map import shard_map
    from concourse.bass2jax import (
        _bass_exec_p,
        install_neuronx_cc_hook,
        partition_id_tensor,
    )

    install_neuronx_cc_hook()
    nc = _NC
    n_cores = 8
    partition_name = nc.partition_id_tensor.name if nc.partition_id_tensor else None
    in_names, out_names, out_avals, zero_shapes = [], [], [], []
    for alloc in nc.m.functions[0].allocations:
        if not isinstance(alloc, mybir.MemoryLocationSet):
            continue
        name = alloc.memorylocations[0].name
        if alloc.kind == "ExternalInput":
            if name != partition_name:
                in_names.append(name)
        elif alloc.kind == "ExternalOutput":
            shape = tuple(alloc.tensor_shape)
            dtype = mybir.dt.np(alloc.dtype)
            out_names.append(name)
            out_avals.append(jax.core.ShapedArray(shape, dtype))
            zero_shapes.append((shape, dtype))
    n_params = len(in_names)
    all_names = list(in_names) + list(out_names)
    if partition_name is not None:
        all_names.append(partition_name)
    donate = tuple(range(n_params, n_params + len(out_names)))

    def _body(*args):
        operands = list(args)
        if partition_name is not None:
            operands.append(partition_id_tensor())
        outs = _bass_exec_p.bind(
            *operands,
            out_avals=tuple(out_avals),
            in_names=tuple(all_names),
            out_names=tuple(out_names),
            lowering_input_output_aliases=(),
            sim_require_finite=True,
            sim_require_nnan=True,
            nc=nc,
        )
        return tuple(outs)

    devices = jax.devices()[:n_cores]
    mesh = Mesh(np.asarray(devices), ("core",))
    nin = n_params + len(out_names)
    sharded = jax.jit(
        shard_map(
            _body,
            mesh=mesh,
            in_specs=(PartitionSpec("core"),) * nin,
            out_specs=(PartitionSpec("core"),) * len(out_names),
            check_rep=False,
        ),
        donate_argnums=donate,
        keep_unused=True,
    )

    def run(per_core_inputs):
        concat_in = [
            np.concatenate([per_core_inputs[c][nm] for c in range(n_cores)], axis=0)
            for nm in in_names
        ]
        zeros = [
            np.zeros((n_cores * s[0],) + tuple(s[1:]), dt) for s, dt in zero_shapes
        ]
        outs = sharded(*concat_in, *zeros)
        return [
            {
                nm: np.asarray(outs[i]).reshape((n_cores,) + tuple(out_avals[i].shape))[c]
                for i, nm in enumerate(out_names)
            }
            for c in range(n_cores)
        ]

    _RUN = run
    return run


def _pack_core(c, hidden, w):
    hg = c % 2
    b = c // 2
    h0 = 2 * hg
    rows = slice(hg * 512, (hg + 1) * 512)
    bfb = np.empty(NBF, BF)

    def put(name, arr):
        o, sz = SEC_BF[name]
        a = np.ascontiguousarray(arr, dtype=np.float32)
        assert a.size == sz, (name, a.shape, sz)
        bfb[o : o + sz] = a.astype(BF).reshape(-1)

    put("hT", hidden[b].T)
    put("wq", w["Wq"][rows].T)
    put("wk", w["Wk"][rows].T)
    put("wv", w["Wv"][rows].T)
    put("wb", w["Wb"][h0 : h0 + 2].T)
    put("w1h", w["W1h"].T)
    put("w1s", w["w1s_dup"])
    put("w2", w["w2_scaled"][:, h0 : h0 + 2, :])
    put("wo", w["Wof"][:, rows].T)

    f32v = np.zeros(NF, np.float32)

    def putf(name, arr):
        o, sz = SEC_F[name]
        a = np.ascontiguousarray(arr, dtype=np.float32)
        assert a.size == sz, (name, a.shape, sz)
        f32v[o : o + sz] = a.reshape(-1)

    cwp = np.empty((128, 4, 3, 4), np.float32)
    for pj, key in enumerate(("conv_q_w", "conv_k_w", "conv_v_w")):
        cc = np.asarray(w[key], np.float32)[rows]  # (512, 4)
        cwp[:, :, pj, :] = cc.reshape(4, 128, 4).transpose(1, 0, 2)
    putf("cw", cwp)
    fir = np.empty((128, 4, FIRS + FIRL), np.float32)
    fir[:, :, :FIRS] = w["fs"][rows].reshape(4, 128, FIRS).transpose(1, 0, 2)
    fir[:, :, FIRS:] = w["fl"][rows].reshape(4, 128, FIRL).transpose(1, 0, 2)
    putf("fir", fir)
    putf("msk", w["msk"])
    putf("b1", np.asarray(w["gate_b1"], np.float32).reshape(8, 128).T)
    putf("b2", w["b2_scaled"][:, h0 : h0 + 2])
    return {"bfb": bfb, "f32b": f32v}


def _prep_shared(inputs):
    w = {}
    for k in ("Wq", "Wk", "Wv", "Wb", "conv_q_w", "conv_k_w", "conv_v_w", "gate_b1"):
        w[k] = np.asarray(inputs[k], np.float32)
    W1 = np.asarray(inputs["gate_W1"], np.float32)
    w["W1h"] = W1[:, :HS]
    W1s = W1[:, HS:].copy()
    for blk in range(4):
        W1s[:, blk * 4 + 0] *= 1.0 / 256.0
        W1s[:, blk * 4 + 2] *= 1.0 / 256.0
    w1sT = W1s.T  # (16, 1024)
    dup = np.zeros((64, 1024), np.float32)
    dup[0:16] = w1sT
    dup[32:48] = w1sT
    w["w1s_dup"] = dup
    temp = np.exp(np.asarray(inputs["gate_log_temp"], np.float32))  # (NH,)
    W2 = np.asarray(inputs["gate_W2"], np.float32)  # (4, 1024)
    w2s = np.empty((HS, NH, 4), np.float32)
    for h in range(NH):
        w2s[:, h, :] = (W2 / temp[h]).T
    w["w2_scaled"] = w2s
    b2 = np.asarray(inputs["gate_b2"], np.float32)
    bias_val = np.asarray(inputs["gate_copy_bias"], np.float32) * DECAY
    b2s = np.empty((4, NH), np.float32)
    for h in range(NH):
        b2s[:, h] = b2 / temp[h]
        b2s[3, h] += bias_val[h] / temp[h]
    w["b2_scaled"] = b2s
    onw = np.asarray(inputs["o_norm_w"], np.float32)
    w["Wof"] = np.asarray(inputs["Wo"], np.float32) * np.tile(onw, NH)[None, :]
    w["fs"] = np.asarray(inputs["fir_short_filt"], np.float32).reshape(NH * DV, FIRS)
    w["fl"] = np.asarray(inputs["fir_long_filt"], np.float32).reshape(NH * DV, FIRL)
    msk = np.zeros((128, 256), np.float32)
    ii = np.arange(128)
    msk[:, :128][ii[:, None] > ii[None, :]] = -1.0  # strict lower
    msk[:, 128:][ii[:, None] <= ii[None, :]] = 1.0  # upper incl diag (attnT)
    w["msk"] = msk
    return w


def _ensure_ready():
    global _NC, _RUN
    if _RUN is None:
        _NC = _build_nc()
        _make_runner()


def _warmup():
    _ensure_ready()
    per_core = [
        {"bfb": np.zeros(NBF, BF), "f32b": np.zeros(NF, np.float32)} for _ in range(8)
    ]
    _RUN(per_core)


def kernel(**inputs):
    _ensure_ready()
    hidden = np.asarray(inputs["hidden_states"], np.float32)
    w = _prep_shared(inputs)
    per_core = [_pack_core(c, hidden, w) for c in range(8)]
    res = _RUN(per_core)
    out = np.zeros((B, L, HS), np.float32)
    for c in range(8):
        out[c // 2] += np.asarray(res[c]["out"]).astype(np.float32).T
    return out


import os as _os

if not _os.environ.get("KERNEL_NO_WARMUP"):
    try:
        _warmup()
    except Exception as _e:  # pragma: no cover
        import traceback

        traceback.print_exc()


# revision 24
# speedup vs baseline: 30.7796x; 1.4077x over previous
"""Fused DeltaNet forward on 8 NeuronCores.

Sharding: core c handles batch b=c//2 and head-pair hg=c%2 (heads 2hg, 2hg+1).
The entire module (projections, causal conv+silu, l2norm, chunkwise delta rule
with chunk=128 via Neumann-doubling triangular inverse, FIR branches, stats,
gate MLP, softmax gate, rmsnorm, partial output projection) runs on-device in
one Bass/Tile program. The host only packs inputs (bf16) and sums the two
head-pair partial outputs per batch.
"""

import sys

sys.path.insert(0, "/opt/trn_rl_repo")

import numpy as np
import ml_dtypes

import concourse.bass as bass
import concourse.bacc as bacc
import concourse.tile as tile
from concourse import mybir
from concourse.bass import ds
from concourse.masks import make_identity

BF = ml_dtypes.bfloat16
F32 = mybir.dt.float32
BF16 = mybir.dt.bfloat16

# Problem constants
B, L, HS = 4, 4096, 1024
NH, DK, DV = 4, 256, 256
CONV, FIRS, FIRL = 4, 5, 64
CHUNK = 128
NCH = L // CHUNK          # 32 chunks
TTS = L // 512            # 8 t-tiles of 512
Q0 = 3                    # zero-pad columns at the head of q/k tiles
DECAY = 1.0 - 1.0 / 3000.0
EPS_FLOOR = 0.08 * DECAY
RMS_EPS = 1e-05
AFT = mybir.ActivationFunctionType
ALU = mybir.AluOpType

# --- bf16 blob layout (per core) ---
SEC_BF = {}
_o = 0
for name, sz in [
    ("hT", 512 * L),         # my half of hidden[b].T (512, 4096)
    ("wq", HS * 512),        # Wq[rows].T  (1024, 512)
    ("wk", HS * 512),
    ("wv", HS * 512),
    ("wb", HS * 2),          # Wb[h0:h0+2].T (1024, 2)
    ("w1h", HS * 1024),      # W1h.T (1024, 1024)
    ("w1s", 64 * 1024),      # W1s.T duplicated rows at 0-15 and 32-47 (64, 1024)
    ("w2", HS * 2 * 4),      # W2.T scaled per head (1024, 2, 4)
    ("wo", 512 * 1024),      # Wo'[:, rows].T (512, 1024)
]:
    SEC_BF[name] = (_o, sz)
    _o += sz
NBF = _o

SEC_F = {}
_o = 0
for name, sz in [
    ("cw", 128 * 4 * 3 * 4),    # conv taps   [p, ftile, proj, tap]
    ("fir", 128 * 4 * (FIRS + FIRL)),  # fir taps [p, ftile, 0:5 fs | 5:69 fl]
    ("msk", 128 * 256),         # [:, :128] strict-lower(-1); [:, 128:] upper-incl(+1)
    ("b1", 128 * 8),            # gate_b1 [p, f1tile]
    ("b2", 4 * 2),              # exp bias per head [j, h]
]:
    SEC_F[name] = (_o, sz)
    _o += sz
NF = _o


def _bfs(blob, name, shape_str, **axes):
    o, sz = SEC_BF[name]
    return blob[ds(o, sz)].rearrange(shape_str, **axes)


def _fs(blob, name, shape_str, **axes):
    o, sz = SEC_F[name]
    return blob[ds(o, sz)].rearrange(shape_str, **axes)


def _build_nc():
    import contextlib

    nc = bacc.Bacc(num_devices=8)
    bfb = nc.declare_dram_parameter("bfb", [NBF], BF16, isOutput=False)
    f32b = nc.declare_dram_parameter("f32b", [NF], F32, isOutput=False)
    out_d = nc.declare_dram_parameter("out", [512, L], BF16, isOutput=True)
    hTfull = nc.dram_tensor("hTfull", [HS * L], BF16, kind="Internal")
    opart = nc.dram_tensor("opart", [HS * L], BF16, kind="Internal")
    PAIRS = [[0, 1], [2, 3], [4, 5], [6, 7]]

    hT_ap = hTfull.rearrange("(kt p t) -> p kt t", p=128, t=L)
    out_ap = opart.rearrange("(ot p t) -> p ot t", p=128, t=L)
    o_wq, _ = SEC_BF["wq"]
    wqkv_ap = bfb[ds(o_wq, 3 * HS * 512)].rearrange(
        "(j kt p n) -> p kt j n", j=3, kt=8, p=128
    )

    with tile.TileContext(nc, pool_alloc_mode="queue") as tc:
        root = contextlib.ExitStack()
        with root:
            o_hT, sz_hT = SEC_BF["hT"]
            hThalf = nc.dram_tensor("hThalf", [sz_hT], BF16, kind="Internal")
            nc.gpsimd.dma_start(out=hThalf[:], in_=bfb[ds(o_hT, sz_hT)])
            nc.gpsimd.collective_compute(
                kind="AllGather",
                op=ALU.bypass,
                replica_groups=PAIRS,
                ins=[hThalf[:]],
                outs=[hTfull[:]],
            )
            consts = root.enter_context(tc.tile_pool(name="consts", bufs=1))
            id_f32 = consts.tile([128, 128], F32, tag="idf")
            make_identity(nc, id_f32)
            ones_f = consts.tile([128, 128], F32, tag="onesf")
            nc.gpsimd.memset(ones_f, 1.0)
            ones_b = consts.tile([128, 1], BF16, tag="onesb")
            nc.gpsimd.memset(ones_b, 1.0)
            msk = consts.tile([128, 256], F32, tag="msk")
            nc.sync.dma_start(out=msk, in_=_fs(f32b, "msk", "(p n) -> p n", p=128))
            cw = consts.tile([128, 4, 3, 4], F32, tag="cw")
            nc.sync.dma_start(
                out=cw, in_=_fs(f32b, "cw", "(p a b c) -> p a b c", p=128, a=4, b=3)
            )
            fw = consts.tile([128, 4, FIRS + FIRL], F32, tag="fw")
            nc.sync.dma_start(
                out=fw, in_=_fs(f32b, "fir", "(p a k) -> p a k", p=128, a=4)
            )
            b1t = consts.tile([128, 8], F32, tag="b1")
            nc.sync.dma_start(out=b1t, in_=_fs(f32b, "b1", "(p a) -> p a", p=128))
            b2t = consts.tile([4, 2], F32, tag="b2")
            nc.sync.dma_start(out=b2t, in_=_fs(f32b, "b2", "(p a) -> p a", p=4))
            epsc = consts.tile([128, 2], F32, tag="epsc")
            nc.gpsimd.memset(epsc[:, 0:1], 1e-6)
            nc.gpsimd.memset(epsc[:, 1:2], RMS_EPS)
            bT = consts.tile([128, NCH, 2], F32, tag="bT")
            S = consts.tile([128, 2, 2, 256], F32, tag="S")
            nc.gpsimd.memset(S, 0.0)
            stats = consts.tile([64, TTS, 512], BF16, tag="stats")
            w1s = consts.tile([64, 1024], BF16, tag="w1s")
            nc.sync.dma_start(out=w1s, in_=_bfs(bfb, "w1s", "(p n) -> p n", p=64))

            vpool = root.enter_context(tc.tile_pool(name="vpool", bufs=1))
            vt = vpool.tile([128, 4, 64 + L], BF16, tag="v")
            nc.vector.memset(vt[:, :, 0:64], 0.0)

            qkpool = root.enter_context(tc.tile_pool(name="qkpool", bufs=1))
            qt = qkpool.tile([128, 4, Q0 + L], BF16, tag="q")
            kt = qkpool.tile([128, 4, Q0 + L], BF16, tag="k")
            nc.vector.memset(qt[:, :, 0:Q0], 0.0)
            nc.vector.memset(kt[:, :, 0:Q0], 0.0)

            # ---------------- Stage A: projections + conv(inplace) + silu + l2norm
            with contextlib.ExitStack() as sa:
                pa = sa.enter_context(tc.tile_pool(name="pa", bufs=1))
                dbl = sa.enter_context(tc.tile_pool(name="dbl", bufs=2))
                psA = sa.enter_context(tc.tile_pool(name="psA", bufs=2, space="PSUM"))
                psR = sa.enter_context(tc.tile_pool(name="psR", bufs=2, space="PSUM"))
                psB = sa.enter_context(tc.tile_pool(name="psB", bufs=2, space="PSUM"))

                wsl = pa.tile([128, 8, 3, 512], BF16, tag="wqkv")
                for _j, _wn in enumerate(("wq", "wk", "wv")):
                    nc.sync.dma_start(
                        out=wsl[:, :, _j, :],
                        in_=_bfs(bfb, _wn, "(kt p n) -> p kt n", p=128, n=512),
                    )
                wbsl = pa.tile([128, 8, 2], BF16, tag="wb")
                nc.sync.dma_start(
                    out=wbsl, in_=_bfs(bfb, "wb", "(kt p n) -> p kt n", p=128, n=2)
                )
                dests = [(qt, Q0), (kt, Q0), (vt, 64)]
                for tt in range(TTS):
                    hsl = dbl.tile([128, 8, 512], BF16, tag="hsl")
                    nc.gpsimd.dma_start(out=hsl, in_=hT_ap[:, :, ds(tt * 512, 512)])
                    for pj in range(3):
                        dest, dofs = dests[pj]
                        for ft in range(4):
                            ps = psA.tile([128, 512], F32, tag="psA")
                            for g in range(8):
                                nc.tensor.matmul(
                                    ps,
                                    wsl[:, g, pj, ds(ft * 128, 128)],
                                    hsl[:, g, :],
                                    start=(g == 0),
                                    stop=(g == 7),
                                )
                            nc.vector.tensor_copy(
                                out=dest[:, ft, ds(dofs + tt * 512, 512)], in_=ps
                            )
                    # beta
                    psb = psR.tile([2, 512], F32, tag="psb")
                    for g in range(8):
                        nc.tensor.matmul(
                            psb, wbsl[:, g, :], hsl[:, g, :],
                            start=(g == 0), stop=(g == 7),
                        )
                    brow = dbl.tile([2, 512], F32, tag="brow")
                    nc.scalar.activation(out=brow, in_=psb, func=AFT.Sigmoid)
                    for g2 in range(4):
                        pst = psR.tile([128, 2], F32, tag="psb")
                        nc.tensor.transpose(
                            pst, brow[0:2, ds(g2 * 128, 128)], id_f32[0:2, 0:2]
                        )
                        nc.vector.tensor_copy(out=bT[:, tt * 4 + g2, :], in_=pst)

                # conv in place (right-to-left t-tiles), then silu in place
                for pj in range(3):
                    dest, dofs = dests[pj]
                    for ft in range(4):
                        for tt in reversed(range(TTS)):
                            acc = dbl.tile([128, 512], F32, tag="cacc")
                            nc.vector.tensor_scalar_mul(
                                acc,
                                dest[:, ft, ds(dofs + tt * 512, 512)],
                                cw[:, ft, pj, 3:4],
                            )
                            for k in range(3):
                                nc.vector.scalar_tensor_tensor(
                                    out=acc,
                                    in0=dest[:, ft, ds(dofs - 3 + k + tt * 512, 512)],
                                    scalar=cw[:, ft, pj, ds(k, 1)],
                                    in1=acc,
                                    op0=ALU.mult,
                                    op1=ALU.add,
                                )
                            nc.scalar.activation(
                                out=dest[:, ft, ds(dofs + tt * 512, 512)],
                                in_=acc,
                                func=AFT.Silu,
                            )

                # l2norm q, k per head
                for x, dofs in ((qt, Q0), (kt, Q0)):
                    for h in range(2):
                        for tt in range(TTS):
                            xsq = dbl.tile([128, 2, 512], BF16, tag="xsq")
                            for f2 in range(2):
                                nc.scalar.square(
                                    xsq[:, f2, :],
                                    x[:, 2 * h + f2, ds(dofs + tt * 512, 512)],
                                )
                            ps = psR.tile([1, 512], F32, tag="psl2")
                            nc.tensor.matmul(
                                ps, ones_b, xsq[:, 0, :], start=True, stop=False
                            )
                            nc.tensor.matmul(
                                ps, ones_b, xsq[:, 1, :], start=False, stop=True
                            )
                            srow = dbl.tile([1, 512], F32, tag="srow")
                            nc.scalar.activation(
                                out=srow, in_=ps, func=AFT.Sqrt, bias=epsc[0:1, 0:1]
                            )
                            nc.vector.reciprocal(srow, srow)
                            rb = psB.tile([128, 512], F32, tag="psBb")
                            nc.tensor.matmul(
                                rb, ones_f[0:1, :], srow, start=True, stop=True
                            )
                            for f2 in range(2):
                                sl = x[:, 2 * h + f2, ds(dofs + tt * 512, 512)]
                                nc.vector.tensor_mul(sl, sl, rb)

            # ---------------- Stage B: delta rule, chunk=128
            odpool = root.enter_context(tc.tile_pool(name="odpool", bufs=1))
            od = odpool.tile([128, 4, L], BF16, tag="od")

            with contextlib.ExitStack() as sb:
                dl = sb.enter_context(tc.tile_pool(name="dl", bufs=2))
                ps1 = sb.enter_context(tc.tile_pool(name="ps1", bufs=3, space="PSUM"))
                ps2 = sb.enter_context(tc.tile_pool(name="ps2", bufs=3, space="PSUM"))
                psD = sb.enter_context(tc.tile_pool(name="psD", bufs=1, space="PSUM"))

                for ci in range(NCH):
                    csl = ds(Q0 + ci * CHUNK, CHUNK)
                    vsl = ds(64 + ci * CHUNK, CHUNK)
                    osl = ds(ci * CHUNK, CHUNK)
                    for h in range(2):
                        bcol = bT[:, ci, ds(h, 1)]
                        qf = dl.tile([128, 256], F32, tag="qf")
                        nc.vector.tensor_copy(out=qf, in_=qt[:, ds(2 * h, 2), csl])
                        kf = dl.tile([128, 256], F32, tag="kf")
                        nc.vector.tensor_copy(out=kf, in_=kt[:, ds(2 * h, 2), csl])
                        vf = dl.tile([128, 256], F32, tag="vf")
                        nc.vector.tensor_copy(out=vf, in_=vt[:, ds(2 * h, 2), vsl])
                        kcdk = dl.tile([128, 256], F32, tag="kcdk")
                        vcdv = dl.tile([128, 256], F32, tag="vcdv")
                        for f2 in range(2):
                            pt = ps1.tile([128, 128], F32, tag="ps1")
                            nc.tensor.transpose(pt, kf[:, ds(f2 * 128, 128)], id_f32)
                            nc.vector.tensor_copy(out=kcdk[:, ds(f2 * 128, 128)], in_=pt)
                            pt2 = ps1.tile([128, 128], F32, tag="ps1")
                            nc.tensor.transpose(pt2, vf[:, ds(f2 * 128, 128)], id_f32)
                            nc.vector.tensor_copy(out=vcdv[:, ds(f2 * 128, 128)], in_=pt2)
                        nc.vector.tensor_scalar_mul(vcdv, vcdv, bcol)
                        kb = dl.tile([128, 256], F32, tag="kb")
                        nc.vector.tensor_scalar_mul(kb, kcdk, bcol)
                        # A
                        aps = ps1.tile([128, 128], F32, tag="ps1")
                        nc.tensor.matmul(
                            aps, kf[:, 0:128], kf[:, 0:128], start=True, stop=False
                        )
                        nc.tensor.matmul(
                            aps, kf[:, 128:256], kf[:, 128:256], start=False, stop=True
                        )
                        A = dl.tile([128, 128], F32, tag="A")
                        nc.vector.scalar_tensor_tensor(
                            out=A, in0=aps, scalar=bcol, in1=msk[:, 0:128],
                            op0=ALU.mult, op1=ALU.mult,
                        )
                        atp = ps1.tile([128, 128], F32, tag="ps1")
                        nc.tensor.transpose(atp, A, id_f32)
                        PT = dl.tile([128, 128], F32, tag="PT")
                        nc.vector.tensor_copy(out=PT, in_=atp)
                        X = dl.tile([128, 128], F32, tag="X")
                        nc.vector.tensor_add(out=X, in0=atp, in1=id_f32)
                        P = A
                        for lvl in range(6):
                            pps = ps1.tile([128, 128], F32, tag="ps1")
                            nc.tensor.matmul(pps, PT, P, start=True, stop=True)
                            if lvl < 5:
                                Pn = dl.tile([128, 128], F32, tag="Pn")
                                nc.vector.tensor_copy(out=Pn, in_=pps)
                            IP = dl.tile([128, 128], F32, tag="IP")
                            nc.vector.tensor_add(out=IP, in0=pps, in1=id_f32)
                            xps = ps1.tile([128, 128], F32, tag="ps1")
                            nc.tensor.matmul(xps, IP, X, start=True, stop=True)
                            Xn = dl.tile([128, 128], F32, tag="X")
                            nc.vector.tensor_copy(out=Xn, in_=xps)
                            X = Xn
                            if lvl < 5:
                                ptp = ps1.tile([128, 128], F32, tag="ps1")
                                nc.tensor.transpose(ptp, Pn, id_f32)
                                PTn = dl.tile([128, 128], F32, tag="PT")
                                nc.vector.tensor_copy(out=PTn, in_=ptp)
                                P, PT = Pn, PTn
                        ups = ps2.tile([128, 256], F32, tag="ps2")
                        nc.tensor.matmul(ups, X, vcdv, start=True, stop=True)
                        u = dl.tile([128, 256], F32, tag="u")
                        nc.vector.tensor_copy(out=u, in_=ups)
                        wps = ps2.tile([128, 256], F32, tag="ps2")
                        nc.tensor.matmul(wps, X, kb, start=True, stop=True)
                        w = dl.tile([128, 256], F32, tag="w")
                        nc.vector.tensor_copy(out=w, in_=wps)
                        wT = dl.tile([128, 2, 128], F32, tag="wT")
                        for f2 in range(2):
                            tp = ps1.tile([128, 128], F32, tag="ps1")
                            nc.tensor.transpose(tp, w[:, ds(f2 * 128, 128)], id_f32)
                            nc.vector.tensor_copy(out=wT[:, f2, :], in_=tp)
                        ats = ps1.tile([128, 128], F32, tag="ps1")
                        nc.tensor.matmul(
                            ats, kf[:, 0:128], qf[:, 0:128], start=True, stop=False
                        )
                        nc.tensor.matmul(
                            ats, kf[:, 128:256], qf[:, 128:256], start=False, stop=True
                        )
                        at = dl.tile([128, 128], F32, tag="at")
                        nc.vector.tensor_mul(at, ats, msk[:, 128:256])
                        # sequential
                        wsp = ps2.tile([128, 256], F32, tag="ps2")
                        nc.tensor.matmul(
                            wsp, wT[:, 0, :], S[:, h, 0, :], start=True, stop=False
                        )
                        nc.tensor.matmul(
                            wsp, wT[:, 1, :], S[:, h, 1, :], start=False, stop=True
                        )
                        nc.vector.tensor_sub(u, u, wsp)
                        osp = ps2.tile([128, 256], F32, tag="ps2")
                        nc.tensor.matmul(
                            osp, qf[:, 0:128], S[:, h, 0, :], start=True, stop=False
                        )
                        nc.tensor.matmul(
                            osp, qf[:, 128:256], S[:, h, 1, :], start=False, stop=False
                        )
                        nc.tensor.matmul(osp, at, u, start=False, stop=True)
                        dsp = psD.tile([128, 2, 256], F32, tag="psD")
                        nc.tensor.matmul(
                            dsp[:, 0, :], kcdk[:, 0:128], u, start=True, stop=True
                        )
                        nc.tensor.matmul(
                            dsp[:, 1, :], kcdk[:, 128:256], u, start=True, stop=True
                        )
                        nc.vector.tensor_add(S[:, h, 0, :], S[:, h, 0, :], dsp[:, 0, :])
                        nc.vector.tensor_add(S[:, h, 1, :], S[:, h, 1, :], dsp[:, 1, :])
                        ob = dl.tile([128, 256], F32, tag="ob")
                        nc.vector.tensor_copy(out=ob, in_=osp)
                        for f2 in range(2):
                            otp = ps1.tile([128, 128], F32, tag="ps1")
                            nc.tensor.transpose(otp, ob[:, ds(f2 * 128, 128)], id_f32)
                            nc.vector.tensor_copy(out=od[:, 2 * h + f2, osl], in_=otp)

            # helper: recompute fir_s slice (5-tap) into dst
            def emit_fir_s(dst, ft, t0, n):
                nc.vector.tensor_scalar_mul(
                    dst, vt[:, ft, ds(64 + t0, n)], fw[:, ft, 4:5]
                )
                for k in range(4):
                    nc.vector.scalar_tensor_tensor(
                        out=dst,
                        in0=vt[:, ft, ds(60 + k + t0, n)],
                        scalar=fw[:, ft, ds(k, 1)],
                        in1=dst,
                        op0=ALU.mult,
                        op1=ALU.add,
                    )

            # ---------------- Stage C: FIR long (into qt) + stats
            flt = qt  # reuse q's SBUF; fir_l values live at [:, ft, Q0 + t]

            with contextlib.ExitStack() as sc:
                cs = sc.enter_context(tc.tile_pool(name="cs", bufs=2))
                cs1 = sc.enter_context(tc.tile_pool(name="cs1", bufs=1))
                psS = sc.enter_context(tc.tile_pool(name="psS", bufs=3, space="PSUM"))

                for ft in range(4):
                    facc = cs1.tile([128, L], F32, tag="facc")
                    nc.vector.tensor_scalar_mul(
                        facc, vt[:, ft, ds(64, L)], fw[:, ft, ds(5 + 63, 1)]
                    )
                    for k in range(63):
                        nc.vector.scalar_tensor_tensor(
                            out=facc,
                            in0=vt[:, ft, ds(1 + k, L)],
                            scalar=fw[:, ft, ds(5 + k, 1)],
                            in1=facc,
                            op0=ALU.mult,
                            op1=ALU.add,
                        )
                    nc.vector.tensor_copy(out=flt[:, ft, ds(Q0, L)], in_=facc)

                for h in range(2):
                    for tt in range(TTS):
                        for src in range(4):
                            xa = cs.tile([128, 2, 512], BF16, tag="xa")
                            xq = cs.tile([128, 2, 512], BF16, tag="xq")
                            if src == 0:
                                fsr = cs1.tile([128, 2, 512], F32, tag="fsr")
                                for f2 in range(2):
                                    emit_fir_s(fsr[:, f2, :], 2 * h + f2, tt * 512, 512)

                            def srcsl(f2):
                                if src == 0:
                                    return fsr[:, f2, :]
                                if src == 1:
                                    return flt[:, 2 * h + f2, ds(Q0 + tt * 512, 512)]
                                if src == 2:
                                    return od[:, 2 * h + f2, ds(tt * 512, 512)]
                                return vt[:, 2 * h + f2, ds(64 + tt * 512, 512)]

                            for f2 in range(2):
                                nc.scalar.activation(
                                    out=xa[:, f2, :], in_=srcsl(f2), func=AFT.Abs
                                )
                                nc.scalar.square(xq[:, f2, :], srcsl(f2))
                            pS = psS.tile([1, 512], F32, tag="psS")
                            pQ = psS.tile([1, 512], F32, tag="psS")
                            pA = psS.tile([1, 512], F32, tag="psS")
                            for f2 in range(2):
                                st, sp = (f2 == 0), (f2 == 1)
                                if src == 0:
                                    nc.tensor.matmul(
                                        pS, ones_f[:, 0:1], srcsl(f2), start=st, stop=sp
                                    )
                                else:
                                    nc.tensor.matmul(
                                        pS, ones_b, srcsl(f2), start=st, stop=sp
                                    )
                                nc.tensor.matmul(
                                    pQ, ones_b, xq[:, f2, :], start=st, stop=sp
                                )
                                nc.tensor.matmul(
                                    pA, ones_b, xa[:, f2, :], start=st, stop=sp
                                )
                            b4 = src * 4
                            r0 = cs.tile([1, 512], BF16, tag="r0")
                            r1 = cs.tile([1, 512], BF16, tag="r1")
                            r2 = cs.tile([1, 512], BF16, tag="r2")
                            r3 = cs.tile([1, 512], BF16, tag="r3")
                            nc.vector.tensor_copy(out=r0, in_=pS)
                            tmp = cs.tile([1, 512], F32, tag="tmpr")
                            nc.scalar.square(tmp, pS)
                            nc.vector.tensor_scalar_mul(r1, pQ, 1.0 / 256.0)
                            nc.vector.scalar_tensor_tensor(
                                out=r1, in0=tmp, scalar=-1.0 / 65536.0,
                                in1=r1, op0=ALU.mult, op1=ALU.add,
                            )
                            nc.scalar.activation(out=r3, in_=pQ, func=AFT.Sqrt)
                            nc.vector.tensor_copy(out=r2, in_=pA)
                            for ri, rr in enumerate((r0, r1, r2, r3)):
                                nc.gpsimd.dma_start(
                                    out=stats[ds(h * 32 + b4 + ri, 1), tt, :],
                                    in_=rr,
                                )

            # ---------------- Stage D: gate MLP + combine + rms + Wo partial
            with contextlib.ExitStack() as sd:
                gs = sd.enter_context(tc.tile_pool(name="gs", bufs=1))
                gh = sd.enter_context(tc.tile_pool(name="gh", bufs=2))
                gr = sd.enter_context(tc.tile_pool(name="gr", bufs=2))
                psG = sd.enter_context(tc.tile_pool(name="psG", bufs=3, space="PSUM"))
                psL = sd.enter_context(tc.tile_pool(name="psL", bufs=3, space="PSUM"))
                psW = sd.enter_context(tc.tile_pool(name="psW", bufs=2, space="PSUM"))

                # weights live in kt's (now dead) SBUF
                ktf = kt.rearrange("p a t -> p (a t)")
                w1h = ktf[:, ds(0, 8192)].rearrange("p (g n) -> p g n", n=1024)
                nc.sync.dma_start(
                    out=w1h, in_=_bfs(bfb, "w1h", "(kt p n) -> p kt n", p=128, n=1024)
                )
                wo = ktf[:, ds(8192, 4096)].rearrange("p (g n) -> p g n", n=1024)
                nc.sync.dma_start(
                    out=wo, in_=_bfs(bfb, "wo", "(kt p n) -> p kt n", p=128, n=1024)
                )
                w2 = ktf[:, ds(12288, 64)].rearrange("p (g h j) -> p g h j", h=2, j=4)
                nc.sync.dma_start(
                    out=w2,
                    in_=_bfs(bfb, "w2", "(kt p h j) -> p kt h j", p=128, h=2, j=4),
                )
                on = ktf[:, ds(12352, 2048)].rearrange("p (a n) -> p a n", n=512)

                for tt in range(TTS):
                    tsl = ds(tt * 512, 512)
                    hsl2 = gh.tile([128, 8, 512], BF16, tag="hsl2")
                    nc.gpsimd.dma_start(out=hsl2, in_=hT_ap[:, :, tsl])
                    for h in range(2):
                        h1 = gs.tile([128, 8, 512], BF16, tag="h1")
                        for f1 in range(8):
                            hp = psG.tile([128, 512], F32, tag="psG")
                            for g in range(8):
                                nc.tensor.matmul(
                                    hp,
                                    w1h[:, g, ds(f1 * 128, 128)],
                                    hsl2[:, g, :],
                                    start=(g == 0),
                                    stop=False,
                                )
                            nc.tensor.matmul(
                                hp,
                                w1s[ds(h * 32, 16), ds(f1 * 128, 128)],
                                stats[ds(h * 32, 16), tt, :],
                                start=False,
                                stop=True,
                            )
                            nc.scalar.activation(
                                out=h1[:, f1, :], in_=hp,
                                func=AFT.Gelu_apprx_tanh, bias=b1t[:, ds(f1, 1)],
                            )
                        lg = psL.tile([4, 512], F32, tag="psL")
                        for g in range(8):
                            nc.tensor.matmul(
                                lg, w2[:, g, h, :], h1[:, g, :],
                                start=(g == 0), stop=(g == 7),
                            )
                        ez = gr.tile([4, 512], F32, tag="r4")
                        nc.scalar.activation(
                            out=ez, in_=lg, func=AFT.Exp, bias=b2t[:, ds(h, 1)]
                        )
                        sps = psL.tile([4, 512], F32, tag="psL")
                        nc.tensor.matmul(
                            sps[0:1, :], ones_f[0:4, 0:1], ez, start=True, stop=True
                        )
                        srec = gr.tile([1, 512], F32, tag="r1")
                        nc.vector.reciprocal(srec, sps[0:1, :])
                        nc.vector.tensor_scalar_mul(srec, srec, 1.0 - 4.0 * EPS_FLOOR)
                        srb = psL.tile([4, 512], F32, tag="psL")
                        nc.tensor.matmul(
                            srb, ones_f[0:1, 0:4], srec, start=True, stop=True
                        )
                        wgt = gr.tile([4, 512], F32, tag="r4")
                        nc.vector.tensor_mul(wgt, ez, srb)
                        nc.vector.tensor_scalar_add(wgt, wgt, EPS_FLOOR)
                        o_acc = gs.tile([128, 2, 512], F32, tag="oacc")
                        for j in range(4):
                            rsp = psL.tile([4, 512], F32, tag="psL")
                            nc.tensor.matmul(
                                rsp[0:1, :], id_f32[0:4, ds(j, 1)], wgt,
                                start=True, stop=True,
                            )
                            rrow = gr.tile([1, 512], F32, tag="r1")
                            nc.vector.tensor_copy(out=rrow, in_=rsp[0:1, :])
                            wb_ = psG.tile([128, 512], F32, tag="psG")
                            nc.tensor.matmul(
                                wb_, ones_f[0:1, :], rrow, start=True, stop=True
                            )
                            for f2 in range(2):
                                if j == 0:
                                    scr = gs.tile([128, 512], F32, tag="scr")
                                    emit_fir_s(scr, 2 * h + f2, tt * 512, 512)
                                    nc.vector.tensor_mul(o_acc[:, f2, :], scr, wb_)
                                else:
                                    if j == 1:
                                        sl = flt[:, 2 * h + f2, ds(Q0 + tt * 512, 512)]
                                    elif j == 2:
                                        sl = od[:, 2 * h + f2, tsl]
                                    else:
                                        sl = vt[:, 2 * h + f2, ds(64 + tt * 512, 512)]
                                    scr = gs.tile([128, 512], F32, tag="scr")
                                    nc.vector.tensor_mul(scr, sl, wb_)
                                    nc.vector.tensor_add(
                                        o_acc[:, f2, :], o_acc[:, f2, :], scr
                                    )
                        # rms over this head's 256 features
                        rps = psL.tile([4, 512], F32, tag="psL")
                        for f2 in range(2):
                            scr = gs.tile([128, 512], F32, tag="scr")
                            nc.scalar.square(scr, o_acc[:, f2, :])
                            nc.tensor.matmul(
                                rps[0:1, :], ones_f[:, 0:1], scr,
                                start=(f2 == 0), stop=(f2 == 1),
                            )
                        rr2 = gr.tile([1, 512], F32, tag="r1")
                        nc.scalar.activation(
                            out=rr2, in_=rps[0:1, :], func=AFT.Sqrt,
                            bias=epsc[0:1, 1:2], scale=1.0 / 256.0,
                        )
                        nc.vector.reciprocal(rr2, rr2)
                        rb2 = psG.tile([128, 512], F32, tag="psG")
                        nc.tensor.matmul(rb2, ones_f[0:1, :], rr2, start=True, stop=True)
                        for f2 in range(2):
                            nc.vector.tensor_mul(
                                on[:, 2 * h + f2, :], o_acc[:, f2, :], rb2
                            )
                    # Wo partial: two halves of 4 of-tiles each
                    for half in range(2):
                        obuf = gs.tile([128, 4, 512], BF16, tag="obuf")
                        for o4 in range(4):
                            of = half * 4 + o4
                            wp = psW.tile([128, 512], F32, tag="psW")
                            for g in range(4):
                                nc.tensor.matmul(
                                    wp,
                                    wo[:, g, ds(of * 128, 128)],
                                    on[:, g, :],
                                    start=(g == 0),
                                    stop=(g == 3),
                                )
                            nc.vector.tensor_copy(out=obuf[:, o4, :], in_=wp)
                        nc.gpsimd.dma_start(
                            out=out_ap[:, ds(half * 4, 4), tsl], in_=obuf
                        )
            osc = nc.dram_tensor("osc", [512 * L], BF16, kind="Internal")
            nc.gpsimd.collective_compute(
                kind="ReduceScatter",
                op=ALU.add,
                replica_groups=PAIRS,
                ins=[opart[:]],
                outs=[osc[:]],
            )
            nc.gpsimd.dma_start(out=out_d[:, :], in_=osc[:])
    nc.compile()
    return nc


# ---------------- host side ----------------

_NC = None
_RUN = None


def _make_runner():
    global _RUN
    import jax
    from jax.sharding import Mesh, PartitionSpec
    from jax.experimental.shard_map import shard_map
    from concourse.bass2jax import (
        _bass_exec_p,
        install_neuronx_cc_hook,
        partition_id_tensor,
    )

    install_neuronx_cc_hook()
    nc = _NC
    n_cores = 8
    partition_name = nc.partition_id_tensor.name if nc.partition_id_tensor else None
    in_names, out_names, out_avals, zero_shapes = [], [], [], []
    for alloc in nc.m.functions[0].allocations:
        if not isinstance(alloc, mybir.MemoryLocationSet):
            continue
        name = alloc.memorylocations[0].name
        if alloc.kind == "ExternalInput":
            if name != partition_name:
                in_names.append(name)
        elif alloc.kind == "ExternalOutput":
            shape = tuple(alloc.tensor_shape)
            dtype = mybir.dt.np(alloc.dtype)
            out_names.append(name)
            out_avals.append(jax.core.ShapedArray(shape, dtype))
            zero_shapes.append((shape, dtype))
    n_params = len(in_names)
    all_names = list(in_names) + list(out_names)
    if partition_name is not None:
        all_names.append(partition_name)
    donate = tuple(range(n_params, n_params + len(out_names)))

    def _body(*args):
        operands = list(args)
        if partition_name is not None:
            operands.append(partition_id_tensor())
        outs = _bass_exec_p.bind(
            *operands,
            out_avals=tuple(out_avals),
            in_names=tuple(all_names),
            out_names=tuple(out_names),
            lowering_input_output_aliases=(),
            sim_require_finite=True,
            sim_require_nnan=True,
            nc=nc,
        )
        return tuple(outs)

    devices = jax.devices()[:n_cores]
    mesh = Mesh(np.asarray(devices), ("core",))
    nin = n_params + len(out_names)
    sharded = jax.jit(
        shard_map(
            _body,
            mesh=mesh,
            in_specs=(PartitionSpec("core"),) * nin,
            out_specs=(PartitionSpec("core"),) * len(out_names),
            check_rep=False,
        ),
        donate_argnums=donate,
        keep_unused=True,
    )

    state = {"donate": None}

    def run(per_core_inputs):
        concat_in = [
            np.concatenate([per_core_inputs[c][nm] for c in range(n_cores)], axis=0)
            for nm in in_names
        ]
        if state["donate"] is not None:
            zeros = state["donate"]
        else:
            zeros = [
                np.zeros((n_cores * s[0],) + tuple(s[1:]), dt) for s, dt in zero_shapes
            ]
        outs = sharded(*concat_in, *zeros)
        # keep device-resident outputs to donate as (ignored) out buffers next call
        state["donate"] = list(outs)
        return [
            {
                nm: np.asarray(outs[i]).reshape((n_cores,) + tuple(out_avals[i].shape))[c]
                for i, nm in enumerate(out_names)
            }
            for c in range(n_cores)
        ]

    _RUN = run
    return run


def _pack_weights_hg(hg, w):
    """All non-hT bf16 sections for one head-pair, as a flat bf16 array."""
    h0 = 2 * hg
    rows = slice(hg * 512, (hg + 1) * 512)
    o_hT, sz_hT = SEC_BF["hT"]
    blob = np.empty(NBF - sz_hT, BF)

    def put(name, arr):
        o, sz = SEC_BF[name]
        o -= sz_hT
        a = np.ascontiguousarray(arr, dtype=np.float32)
        assert a.size == sz, (name, a.shape, sz)
        blob[o : o + sz] = a.astype(BF).reshape(-1)

    put("wq", w["Wq"][rows].T)
    put("wk", w["Wk"][rows].T)
    put("wv", w["Wv"][rows].T)
    put("wb", w["Wb"][h0 : h0 + 2].T)
    put("w1h", w["W1h"].T)
    put("w1s", w["w1s_dup"])
    put("w2", w["w2_scaled"][:, h0 : h0 + 2, :])
    put("wo", w["Wof"][:, rows].T)

    f32v = np.zeros(NF, np.float32)

    def putf(name, arr):
        o, sz = SEC_F[name]
        a = np.ascontiguousarray(arr, dtype=np.float32)
        assert a.size == sz, (name, a.shape, sz)
        f32v[o : o + sz] = a.reshape(-1)

    cwp = np.empty((128, 4, 3, 4), np.float32)
    for pj, key in enumerate(("conv_q_w", "conv_k_w", "conv_v_w")):
        cc = np.asarray(w[key], np.float32)[rows]  # (512, 4)
        cwp[:, :, pj, :] = cc.reshape(4, 128, 4).transpose(1, 0, 2)
    putf("cw", cwp)
    fir = np.empty((128, 4, FIRS + FIRL), np.float32)
    fir[:, :, :FIRS] = w["fs"][rows].reshape(4, 128, FIRS).transpose(1, 0, 2)
    fir[:, :, FIRS:] = w["fl"][rows].reshape(4, 128, FIRL).transpose(1, 0, 2)
    putf("fir", fir)
    putf("msk", w["msk"])
    putf("b1", np.asarray(w["gate_b1"], np.float32).reshape(8, 128).T)
    putf("b2", w["b2_scaled"][:, h0 : h0 + 2])
    return blob, f32v


def _pack_all(hidden, w):
    wblobs = [_pack_weights_hg(hg, w) for hg in range(2)]
    hb = hidden.astype(BF)  # bf16 first, then cheap bf16 transposes
    per_core = []
    for c in range(8):
        b, hg = c // 2, c % 2
        half = hb[b][:, hg * 512 : (hg + 1) * 512].T  # (512g, 4096t)
        bfb = np.concatenate(
            [np.ascontiguousarray(half).reshape(-1), wblobs[hg][0]]
        )
        per_core.append({"bfb": bfb, "f32b": wblobs[hg][1]})
    return per_core


def _prep_shared(inputs):
    w = {}
    for k in ("Wq", "Wk", "Wv", "Wb", "conv_q_w", "conv_k_w", "conv_v_w", "gate_b1"):
        w[k] = np.asarray(inputs[k], np.float32)
    W1 = np.asarray(inputs["gate_W1"], np.float32)
    w["W1h"] = W1[:, :HS]
    W1s = W1[:, HS:].copy()
    for blk in range(4):
        W1s[:, blk * 4 + 0] *= 1.0 / 256.0
        W1s[:, blk * 4 + 2] *= 1.0 / 256.0
    w1sT = W1s.T  # (16, 1024)
    dup = np.zeros((64, 1024), np.float32)
    dup[0:16] = w1sT
    dup[32:48] = w1sT
    w["w1s_dup"] = dup
    temp = np.exp(np.asarray(inputs["gate_log_temp"], np.float32))  # (NH,)
    W2 = np.asarray(inputs["gate_W2"], np.float32)  # (4, 1024)
    w2s = np.empty((HS, NH, 4), np.float32)
    for h in range(NH):
        w2s[:, h, :] = (W2 / temp[h]).T
    w["w2_scaled"] = w2s
    b2 = np.asarray(inputs["gate_b2"], np.float32)
    bias_val = np.asarray(inputs["gate_copy_bias"], np.float32) * DECAY
    b2s = np.empty((4, NH), np.float32)
    for h in range(NH):
        b2s[:, h] = b2 / temp[h]
        b2s[3, h] += bias_val[h] / temp[h]
    w["b2_scaled"] = b2s
    onw = np.asarray(inputs["o_norm_w"], np.float32)
    w["Wof"] = np.asarray(inputs["Wo"], np.float32) * np.tile(onw, NH)[None, :]
    w["fs"] = np.asarray(inputs["fir_short_filt"], np.float32).reshape(NH * DV, FIRS)
    w["fl"] = np.asarray(inputs["fir_long_filt"], np.float32).reshape(NH * DV, FIRL)
    msk = np.zeros((128, 256), np.float32)
    ii = np.arange(128)
    msk[:, :128][ii[:, None] > ii[None, :]] = -1.0  # strict lower
    msk[:, 128:][ii[:, None] <= ii[None, :]] = 1.0  # upper incl diag (attnT)
    w["msk"] = msk
    return w


def _ensure_ready():
    global _NC, _RUN
    if _RUN is None:
        _NC = _build_nc()
        _make_runner()


def _warmup():
    _ensure_ready()
    per_core = [
        {"bfb": np.zeros(NBF, BF), "f32b": np.zeros(NF, np.float32)} for _ in range(8)
    ]
    _RUN(per_core)


def kernel(**inputs):
    _ensure_ready()
    hidden = np.asarray(inputs["hidden_states"], np.float32)
    w = _prep_shared(inputs)
    per_core = _pack_all(hidden, w)
    res = _RUN(per_core)
    out = np.empty((B, L, HS), np.float32)
    for b in range(B):
        full = np.concatenate(
            [np.asarray(res[2 * b]["out"]), np.asarray(res[2 * b + 1]["out"])], axis=0
        )
        out[b] = full.astype(np.float32).T
    return out


import os as _os

if not _os.environ.get("KERNEL_NO_WARMUP"):
    try:
        _warmup()
    except Exception as _e:  # pragma: no cover
        import traceback

        traceback.print_exc()


# revision 25
# speedup vs baseline: 36.1814x; 1.1755x over previous
"""Fused DeltaNet forward on 8 NeuronCores.

Sharding: core c handles batch b=c//2 and head-pair hg=c%2 (heads 2hg, 2hg+1).
The entire module (projections, causal conv+silu, l2norm, chunkwise delta rule
with chunk=128 via Neumann-doubling triangular inverse, FIR branches, stats,
gate MLP, softmax gate, rmsnorm, partial output projection) runs on-device in
one Bass/Tile program. The host only packs inputs (bf16) and sums the two
head-pair partial outputs per batch.
"""

import sys

sys.path.insert(0, "/opt/trn_rl_repo")

import numpy as np
import ml_dtypes

import concourse.bass as bass
import concourse.bacc as bacc
import concourse.tile as tile
from concourse import mybir
from concourse.bass import ds
from concourse.masks import make_identity

BF = ml_dtypes.bfloat16
F32 = mybir.dt.float32
BF16 = mybir.dt.bfloat16

# Problem constants
B, L, HS = 4, 4096, 1024
NH, DK, DV = 4, 256, 256
CONV, FIRS, FIRL = 4, 5, 64
CHUNK = 128
NCH = L // CHUNK          # 32 chunks
TTS = L // 512            # 8 t-tiles of 512
Q0 = 3                    # zero-pad columns at the head of q/k tiles
DECAY = 1.0 - 1.0 / 3000.0
EPS_FLOOR = 0.08 * DECAY
RMS_EPS = 1e-05
AFT = mybir.ActivationFunctionType
ALU = mybir.AluOpType

# --- weight blob layout (per head-pair, AllGathered on device) ---
SEC_W = {}
_o = 0
for name, sz in [
    ("wq", HS * 512),        # Wq[rows].T  (1024, 512)
    ("wk", HS * 512),
    ("wv", HS * 512),
    ("wb", HS * 2),          # Wb[h0:h0+2].T (1024, 2)
    ("w1h", HS * 1024),      # W1h.T (1024, 1024)
    ("w1s", 64 * 1024),      # W1s.T duplicated rows at 0-15 and 32-47 (64, 1024)
    ("w2", HS * 2 * 4),      # W2.T scaled per head (1024, 2, 4)
    ("wo", 512 * 1024),      # Wo'[:, rows].T (512, 1024)
]:
    SEC_W[name] = (_o, sz)
    _o += sz
NW = _o                      # full weight blob elems (per hg variant)
assert NW % 4 == 0
NWQ = NW // 4                # quarter shipped per core
SEC_BF = {"hT": (0, 512 * L), "wqtr": (512 * L, NWQ)}
NBF = 512 * L + NWQ

SEC_F = {}
_o = 0
for name, sz in [
    ("cw", 128 * 4 * 3 * 4),    # conv taps   [p, ftile, proj, tap]
    ("fir", 128 * 4 * (FIRS + FIRL)),  # fir taps [p, ftile, 0:5 fs | 5:69 fl]
    ("msk", 128 * 256),         # [:, :128] strict-lower(-1); [:, 128:] upper-incl(+1)
    ("b1", 128 * 8),            # gate_b1 [p, f1tile]
    ("b2", 4 * 2),              # exp bias per head [j, h]
]:
    SEC_F[name] = (_o, sz)
    _o += sz
NF = _o


def _bfs(blob, name, shape_str, **axes):
    o, sz = SEC_BF[name]
    return blob[ds(o, sz)].rearrange(shape_str, **axes)


def _ws(wfull, name, shape_str, **axes):
    o, sz = SEC_W[name]
    return wfull[ds(o, sz)].rearrange(shape_str, **axes)


def _fs(blob, name, shape_str, **axes):
    o, sz = SEC_F[name]
    return blob[ds(o, sz)].rearrange(shape_str, **axes)


def _build_nc():
    import contextlib

    nc = bacc.Bacc(num_devices=8)
    bfb = nc.declare_dram_parameter("bfb", [NBF], BF16, isOutput=False)
    f32b = nc.declare_dram_parameter("f32b", [NF], F32, isOutput=False)
    out_d = nc.declare_dram_parameter("out", [512, L], BF16, isOutput=True)
    hTfull = nc.dram_tensor("hTfull", [HS * L], BF16, kind="Internal")
    opart = nc.dram_tensor("opart", [HS * L], BF16, kind="Internal")
    PAIRS = [[0, 1], [2, 3], [4, 5], [6, 7]]

    hT_ap = hTfull.rearrange("(kt p t) -> p kt t", p=128, t=L)
    out_ap = opart.rearrange("(ot p t) -> p ot t", p=128, t=L)

    with tile.TileContext(nc, pool_alloc_mode="queue") as tc:
        root = contextlib.ExitStack()
        with root:
            o_hT, sz_hT = SEC_BF["hT"]
            hThalf = nc.dram_tensor("hThalf", [sz_hT], BF16, kind="Internal")
            nc.gpsimd.dma_start(out=hThalf[:], in_=bfb[ds(o_hT, sz_hT)])
            nc.gpsimd.collective_compute(
                kind="AllGather",
                op=ALU.bypass,
                replica_groups=PAIRS,
                ins=[hThalf[:]],
                outs=[hTfull[:]],
            )
            o_wq_, sz_wq_ = SEC_BF["wqtr"]
            wqtr = nc.dram_tensor("wqtr", [NWQ], BF16, kind="Internal")
            nc.gpsimd.dma_start(out=wqtr[:], in_=bfb[ds(o_wq_, sz_wq_)])
            wfull = nc.dram_tensor("wfull", [NW], BF16, kind="Internal")
            nc.gpsimd.collective_compute(
                kind="AllGather",
                op=ALU.bypass,
                replica_groups=[[0, 2, 4, 6], [1, 3, 5, 7]],
                ins=[wqtr[:]],
                outs=[wfull[:]],
            )
            consts = root.enter_context(tc.tile_pool(name="consts", bufs=1))
            id_f32 = consts.tile([128, 128], F32, tag="idf")
            make_identity(nc, id_f32)
            ones_f = consts.tile([128, 128], F32, tag="onesf")
            nc.gpsimd.memset(ones_f, 1.0)
            ones_b = consts.tile([128, 1], BF16, tag="onesb")
            nc.gpsimd.memset(ones_b, 1.0)
            msk = consts.tile([128, 256], F32, tag="msk")
            nc.sync.dma_start(out=msk, in_=_fs(f32b, "msk", "(p n) -> p n", p=128))
            cw = consts.tile([128, 4, 3, 4], F32, tag="cw")
            nc.sync.dma_start(
                out=cw, in_=_fs(f32b, "cw", "(p a b c) -> p a b c", p=128, a=4, b=3)
            )
            fw = consts.tile([128, 4, FIRS + FIRL], F32, tag="fw")
            nc.sync.dma_start(
                out=fw, in_=_fs(f32b, "fir", "(p a k) -> p a k", p=128, a=4)
            )
            b1t = consts.tile([128, 8], F32, tag="b1")
            nc.sync.dma_start(out=b1t, in_=_fs(f32b, "b1", "(p a) -> p a", p=128))
            b2t = consts.tile([4, 2], F32, tag="b2")
            nc.sync.dma_start(out=b2t, in_=_fs(f32b, "b2", "(p a) -> p a", p=4))
            epsc = consts.tile([128, 2], F32, tag="epsc")
            nc.gpsimd.memset(epsc[:, 0:1], 1e-6)
            nc.gpsimd.memset(epsc[:, 1:2], RMS_EPS)
            bT = consts.tile([128, NCH, 2], F32, tag="bT")
            S = consts.tile([128, 2, 2, 256], F32, tag="S")
            nc.gpsimd.memset(S, 0.0)
            stats = consts.tile([64, TTS, 512], BF16, tag="stats")
            w1s = consts.tile([64, 1024], BF16, tag="w1s")
            nc.sync.dma_start(out=w1s, in_=_ws(wfull, "w1s", "(p n) -> p n", p=64))

            vpool = root.enter_context(tc.tile_pool(name="vpool", bufs=1))
            vt = vpool.tile([128, 4, 64 + L], BF16, tag="v")
            nc.vector.memset(vt[:, :, 0:64], 0.0)

            qkpool = root.enter_context(tc.tile_pool(name="qkpool", bufs=1))
            qt = qkpool.tile([128, 4, Q0 + L], BF16, tag="q")
            kt = qkpool.tile([128, 4, Q0 + L], BF16, tag="k")
            nc.vector.memset(qt[:, :, 0:Q0], 0.0)
            nc.vector.memset(kt[:, :, 0:Q0], 0.0)

            # ---------------- Stage A: projections + conv(inplace) + silu + l2norm
            with contextlib.ExitStack() as sa:
                pa = sa.enter_context(tc.tile_pool(name="pa", bufs=1))
                dbl = sa.enter_context(tc.tile_pool(name="dbl", bufs=2))
                psA = sa.enter_context(tc.tile_pool(name="psA", bufs=2, space="PSUM"))
                psR = sa.enter_context(tc.tile_pool(name="psR", bufs=2, space="PSUM"))
                psB = sa.enter_context(tc.tile_pool(name="psB", bufs=2, space="PSUM"))

                wsl = pa.tile([128, 8, 3, 512], BF16, tag="wqkv")
                for _j, _wn in enumerate(("wq", "wk", "wv")):
                    nc.sync.dma_start(
                        out=wsl[:, :, _j, :],
                        in_=_ws(wfull, _wn, "(kt p n) -> p kt n", p=128, n=512),
                    )
                wbsl = pa.tile([128, 8, 2], BF16, tag="wb")
                nc.sync.dma_start(
                    out=wbsl, in_=_ws(wfull, "wb", "(kt p n) -> p kt n", p=128, n=2)
                )
                dests = [(qt, Q0), (kt, Q0), (vt, 64)]
                for tt in range(TTS):
                    hsl = dbl.tile([128, 8, 512], BF16, tag="hsl")
                    nc.gpsimd.dma_start(out=hsl, in_=hT_ap[:, :, ds(tt * 512, 512)])
                    for pj in range(3):
                        dest, dofs = dests[pj]
                        for ft in range(4):
                            ps = psA.tile([128, 512], F32, tag="psA")
                            for g in range(8):
                                nc.tensor.matmul(
                                    ps,
                                    wsl[:, g, pj, ds(ft * 128, 128)],
                                    hsl[:, g, :],
                                    start=(g == 0),
                                    stop=(g == 7),
                                )
                            nc.vector.tensor_copy(
                                out=dest[:, ft, ds(dofs + tt * 512, 512)], in_=ps
                            )
                    # beta
                    psb = psR.tile([2, 512], F32, tag="psb")
                    for g in range(8):
                        nc.tensor.matmul(
                            psb, wbsl[:, g, :], hsl[:, g, :],
                            start=(g == 0), stop=(g == 7),
                        )
                    brow = dbl.tile([2, 512], F32, tag="brow")
                    nc.scalar.activation(out=brow, in_=psb, func=AFT.Sigmoid)
                    for g2 in range(4):
                        pst = psR.tile([128, 2], F32, tag="psb")
                        nc.tensor.transpose(
                            pst, brow[0:2, ds(g2 * 128, 128)], id_f32[0:2, 0:2]
                        )
                        nc.vector.tensor_copy(out=bT[:, tt * 4 + g2, :], in_=pst)

                # conv in place (right-to-left t-tiles), then silu in place
                for pj in range(3):
                    dest, dofs = dests[pj]
                    for ft in range(4):
                        for tt in reversed(range(TTS)):
                            acc = dbl.tile([128, 512], F32, tag="cacc")
                            nc.vector.tensor_scalar_mul(
                                acc,
                                dest[:, ft, ds(dofs + tt * 512, 512)],
                                cw[:, ft, pj, 3:4],
                            )
                            for k in range(3):
                                nc.vector.scalar_tensor_tensor(
                                    out=acc,
                                    in0=dest[:, ft, ds(dofs - 3 + k + tt * 512, 512)],
                                    scalar=cw[:, ft, pj, ds(k, 1)],
                                    in1=acc,
                                    op0=ALU.mult,
                                    op1=ALU.add,
                                )
                            nc.scalar.activation(
                                out=dest[:, ft, ds(dofs + tt * 512, 512)],
                                in_=acc,
                                func=AFT.Silu,
                            )

                # l2norm q, k per head
                for x, dofs in ((qt, Q0), (kt, Q0)):
                    for h in range(2):
                        for tt in range(TTS):
                            xsq = dbl.tile([128, 2, 512], BF16, tag="xsq")
                            for f2 in range(2):
                                nc.scalar.square(
                                    xsq[:, f2, :],
                                    x[:, 2 * h + f2, ds(dofs + tt * 512, 512)],
                                )
                            ps = psR.tile([1, 512], F32, tag="psl2")
                            nc.tensor.matmul(
                                ps, ones_b, xsq[:, 0, :], start=True, stop=False
                            )
                            nc.tensor.matmul(
                                ps, ones_b, xsq[:, 1, :], start=False, stop=True
                            )
                            srow = dbl.tile([1, 512], F32, tag="srow")
                            nc.scalar.activation(
                                out=srow, in_=ps, func=AFT.Sqrt, bias=epsc[0:1, 0:1]
                            )
                            nc.vector.reciprocal(srow, srow)
                            rb = psB.tile([128, 512], F32, tag="psBb")
                            nc.tensor.matmul(
                                rb, ones_f[0:1, :], srow, start=True, stop=True
                            )
                            for f2 in range(2):
                                sl = x[:, 2 * h + f2, ds(dofs + tt * 512, 512)]
                                nc.vector.tensor_mul(sl, sl, rb)

            # ---------------- Stage B: delta rule, chunk=128
            odpool = root.enter_context(tc.tile_pool(name="odpool", bufs=1))
            od = odpool.tile([128, 4, L], BF16, tag="od")

            with contextlib.ExitStack() as sb:
                dl = sb.enter_context(tc.tile_pool(name="dl", bufs=2))
                ps1 = sb.enter_context(tc.tile_pool(name="ps1", bufs=3, space="PSUM"))
                ps2 = sb.enter_context(tc.tile_pool(name="ps2", bufs=3, space="PSUM"))
                psD = sb.enter_context(tc.tile_pool(name="psD", bufs=1, space="PSUM"))

                for ci in range(NCH):
                    csl = ds(Q0 + ci * CHUNK, CHUNK)
                    vsl = ds(64 + ci * CHUNK, CHUNK)
                    osl = ds(ci * CHUNK, CHUNK)
                    for h in range(2):
                        bcol = bT[:, ci, ds(h, 1)]
                        qf = dl.tile([128, 256], F32, tag="qf")
                        nc.vector.tensor_copy(out=qf, in_=qt[:, ds(2 * h, 2), csl])
                        kf = dl.tile([128, 256], F32, tag="kf")
                        nc.vector.tensor_copy(out=kf, in_=kt[:, ds(2 * h, 2), csl])
                        vf = dl.tile([128, 256], F32, tag="vf")
                        nc.vector.tensor_copy(out=vf, in_=vt[:, ds(2 * h, 2), vsl])
                        kcdk = dl.tile([128, 256], F32, tag="kcdk")
                        vcdv = dl.tile([128, 256], F32, tag="vcdv")
                        for f2 in range(2):
                            pt = ps1.tile([128, 128], F32, tag="ps1")
                            nc.tensor.transpose(pt, kf[:, ds(f2 * 128, 128)], id_f32)
                            nc.vector.tensor_copy(out=kcdk[:, ds(f2 * 128, 128)], in_=pt)
                            pt2 = ps1.tile([128, 128], F32, tag="ps1")
                            nc.tensor.transpose(pt2, vf[:, ds(f2 * 128, 128)], id_f32)
                            nc.vector.tensor_copy(out=vcdv[:, ds(f2 * 128, 128)], in_=pt2)
                        nc.vector.tensor_scalar_mul(vcdv, vcdv, bcol)
                        kb = dl.tile([128, 256], F32, tag="kb")
                        nc.vector.tensor_scalar_mul(kb, kcdk, bcol)
                        # A
                        aps = ps1.tile([128, 128], F32, tag="ps1")
                        nc.tensor.matmul(
                            aps, kf[:, 0:128], kf[:, 0:128], start=True, stop=False
                        )
                        nc.tensor.matmul(
                            aps, kf[:, 128:256], kf[:, 128:256], start=False, stop=True
                        )
                        A = dl.tile([128, 128], F32, tag="A")
                        nc.vector.scalar_tensor_tensor(
                            out=A, in0=aps, scalar=bcol, in1=msk[:, 0:128],
                            op0=ALU.mult, op1=ALU.mult,
                        )
                        atp = ps1.tile([128, 128], F32, tag="ps1")
                        nc.tensor.transpose(atp, A, id_f32)
                        PT = dl.tile([128, 128], F32, tag="PT")
                        nc.vector.tensor_copy(out=PT, in_=atp)
                        X = dl.tile([128, 128], F32, tag="X")
                        nc.vector.tensor_add(out=X, in0=atp, in1=id_f32)
                        P = A
                        for lvl in range(6):
                            pps = ps1.tile([128, 128], F32, tag="ps1")
                            nc.tensor.matmul(pps, PT, P, start=True, stop=True)
                            if lvl < 5:
                                Pn = dl.tile([128, 128], F32, tag="Pn")
                                nc.vector.tensor_copy(out=Pn, in_=pps)
                            IP = dl.tile([128, 128], F32, tag="IP")
                            nc.vector.tensor_add(out=IP, in0=pps, in1=id_f32)
                            xps = ps1.tile([128, 128], F32, tag="ps1")
                            nc.tensor.matmul(xps, IP, X, start=True, stop=True)
                            Xn = dl.tile([128, 128], F32, tag="X")
                            nc.vector.tensor_copy(out=Xn, in_=xps)
                            X = Xn
                            if lvl < 5:
                                ptp = ps1.tile([128, 128], F32, tag="ps1")
                                nc.tensor.transpose(ptp, Pn, id_f32)
                                PTn = dl.tile([128, 128], F32, tag="PT")
                                nc.vector.tensor_copy(out=PTn, in_=ptp)
                                P, PT = Pn, PTn
                        ups = ps2.tile([128, 256], F32, tag="ps2")
                        nc.tensor.matmul(ups, X, vcdv, start=True, stop=True)
                        u = dl.tile([128, 256], F32, tag="u")
                        nc.vector.tensor_copy(out=u, in_=ups)
                        wps = ps2.tile([128, 256], F32, tag="ps2")
                        nc.tensor.matmul(wps, X, kb, start=True, stop=True)
                        w = dl.tile([128, 256], F32, tag="w")
                        nc.vector.tensor_copy(out=w, in_=wps)
                        wT = dl.tile([128, 2, 128], F32, tag="wT")
                        for f2 in range(2):
                            tp = ps1.tile([128, 128], F32, tag="ps1")
                            nc.tensor.transpose(tp, w[:, ds(f2 * 128, 128)], id_f32)
                            nc.vector.tensor_copy(out=wT[:, f2, :], in_=tp)
                        ats = ps1.tile([128, 128], F32, tag="ps1")
                        nc.tensor.matmul(
                            ats, kf[:, 0:128], qf[:, 0:128], start=True, stop=False
                        )
                        nc.tensor.matmul(
                            ats, kf[:, 128:256], qf[:, 128:256], start=False, stop=True
                        )
                        at = dl.tile([128, 128], F32, tag="at")
                        nc.vector.tensor_mul(at, ats, msk[:, 128:256])
                        # sequential
                        wsp = ps2.tile([128, 256], F32, tag="ps2")
                        nc.tensor.matmul(
                            wsp, wT[:, 0, :], S[:, h, 0, :], start=True, stop=False
                        )
                        nc.tensor.matmul(
                            wsp, wT[:, 1, :], S[:, h, 1, :], start=False, stop=True
                        )
                        nc.vector.tensor_sub(u, u, wsp)
                        osp = ps2.tile([128, 256], F32, tag="ps2")
                        nc.tensor.matmul(
                            osp, qf[:, 0:128], S[:, h, 0, :], start=True, stop=False
                        )
                        nc.tensor.matmul(
                            osp, qf[:, 128:256], S[:, h, 1, :], start=False, stop=False
                        )
                        nc.tensor.matmul(osp, at, u, start=False, stop=True)
                        dsp = psD.tile([128, 2, 256], F32, tag="psD")
                        nc.tensor.matmul(
                            dsp[:, 0, :], kcdk[:, 0:128], u, start=True, stop=True
                        )
                        nc.tensor.matmul(
                            dsp[:, 1, :], kcdk[:, 128:256], u, start=True, stop=True
                        )
                        nc.vector.tensor_add(S[:, h, 0, :], S[:, h, 0, :], dsp[:, 0, :])
                        nc.vector.tensor_add(S[:, h, 1, :], S[:, h, 1, :], dsp[:, 1, :])
                        ob = dl.tile([128, 256], F32, tag="ob")
                        nc.vector.tensor_copy(out=ob, in_=osp)
                        for f2 in range(2):
                            otp = ps1.tile([128, 128], F32, tag="ps1")
                            nc.tensor.transpose(otp, ob[:, ds(f2 * 128, 128)], id_f32)
                            nc.vector.tensor_copy(out=od[:, 2 * h + f2, osl], in_=otp)

            # helper: recompute fir_s slice (5-tap) into dst
            def emit_fir_s(dst, ft, t0, n):
                nc.vector.tensor_scalar_mul(
                    dst, vt[:, ft, ds(64 + t0, n)], fw[:, ft, 4:5]
                )
                for k in range(4):
                    nc.vector.scalar_tensor_tensor(
                        out=dst,
                        in0=vt[:, ft, ds(60 + k + t0, n)],
                        scalar=fw[:, ft, ds(k, 1)],
                        in1=dst,
                        op0=ALU.mult,
                        op1=ALU.add,
                    )

            # ---------------- Stage C: FIR long (into qt) + stats
            flt = qt  # reuse q's SBUF; fir_l values live at [:, ft, Q0 + t]

            with contextlib.ExitStack() as sc:
                cs = sc.enter_context(tc.tile_pool(name="cs", bufs=2))
                cs1 = sc.enter_context(tc.tile_pool(name="cs1", bufs=1))
                psS = sc.enter_context(tc.tile_pool(name="psS", bufs=3, space="PSUM"))

                for ft in range(4):
                    facc = cs1.tile([128, L], F32, tag="facc")
                    nc.vector.tensor_scalar_mul(
                        facc, vt[:, ft, ds(64, L)], fw[:, ft, ds(5 + 63, 1)]
                    )
                    for k in range(63):
                        nc.vector.scalar_tensor_tensor(
                            out=facc,
                            in0=vt[:, ft, ds(1 + k, L)],
                            scalar=fw[:, ft, ds(5 + k, 1)],
                            in1=facc,
                            op0=ALU.mult,
                            op1=ALU.add,
                        )
                    nc.vector.tensor_copy(out=flt[:, ft, ds(Q0, L)], in_=facc)

                for h in range(2):
                    for tt in range(TTS):
                        for src in range(4):
                            xa = cs.tile([128, 2, 512], BF16, tag="xa")
                            xq = cs.tile([128, 2, 512], BF16, tag="xq")
                            if src == 0:
                                fsr = cs1.tile([128, 2, 512], F32, tag="fsr")
                                for f2 in range(2):
                                    emit_fir_s(fsr[:, f2, :], 2 * h + f2, tt * 512, 512)

                            def srcsl(f2):
                                if src == 0:
                                    return fsr[:, f2, :]
                                if src == 1:
                                    return flt[:, 2 * h + f2, ds(Q0 + tt * 512, 512)]
                                if src == 2:
                                    return od[:, 2 * h + f2, ds(tt * 512, 512)]
                                return vt[:, 2 * h + f2, ds(64 + tt * 512, 512)]

                            for f2 in range(2):
                                nc.scalar.activation(
                                    out=xa[:, f2, :], in_=srcsl(f2), func=AFT.Abs
                                )
                                nc.scalar.square(xq[:, f2, :], srcsl(f2))
                            pS = psS.tile([1, 512], F32, tag="psS")
                            pQ = psS.tile([1, 512], F32, tag="psS")
                            pA = psS.tile([1, 512], F32, tag="psS")
                            for f2 in range(2):
                                st, sp = (f2 == 0), (f2 == 1)
                                if src == 0:
                                    nc.tensor.matmul(
                                        pS, ones_f[:, 0:1], srcsl(f2), start=st, stop=sp
                                    )
                                else:
                                    nc.tensor.matmul(
                                        pS, ones_b, srcsl(f2), start=st, stop=sp
                                    )
                                nc.tensor.matmul(
                                    pQ, ones_b, xq[:, f2, :], start=st, stop=sp
                                )
                                nc.tensor.matmul(
                                    pA, ones_b, xa[:, f2, :], start=st, stop=sp
                                )
                            b4 = src * 4
                            r0 = cs.tile([1, 512], BF16, tag="r0")
                            r1 = cs.tile([1, 512], BF16, tag="r1")
                            r2 = cs.tile([1, 512], BF16, tag="r2")
                            r3 = cs.tile([1, 512], BF16, tag="r3")
                            nc.vector.tensor_copy(out=r0, in_=pS)
                            tmp = cs.tile([1, 512], F32, tag="tmpr")
                            nc.scalar.square(tmp, pS)
                            nc.vector.tensor_scalar_mul(r1, pQ, 1.0 / 256.0)
                            nc.vector.scalar_tensor_tensor(
                                out=r1, in0=tmp, scalar=-1.0 / 65536.0,
                                in1=r1, op0=ALU.mult, op1=ALU.add,
                            )
                            nc.scalar.activation(out=r3, in_=pQ, func=AFT.Sqrt)
                            nc.vector.tensor_copy(out=r2, in_=pA)
                            for ri, rr in enumerate((r0, r1, r2, r3)):
                                nc.gpsimd.dma_start(
                                    out=stats[ds(h * 32 + b4 + ri, 1), tt, :],
                                    in_=rr,
                                )

            # ---------------- Stage D: gate MLP + combine + rms + Wo partial
            with contextlib.ExitStack() as sd:
                gs = sd.enter_context(tc.tile_pool(name="gs", bufs=1))
                gh = sd.enter_context(tc.tile_pool(name="gh", bufs=2))
                gr = sd.enter_context(tc.tile_pool(name="gr", bufs=2))
                psG = sd.enter_context(tc.tile_pool(name="psG", bufs=3, space="PSUM"))
                psL = sd.enter_context(tc.tile_pool(name="psL", bufs=3, space="PSUM"))
                psW = sd.enter_context(tc.tile_pool(name="psW", bufs=2, space="PSUM"))

                # weights live in kt's (now dead) SBUF
                ktf = kt.rearrange("p a t -> p (a t)")
                w1h = ktf[:, ds(0, 8192)].rearrange("p (g n) -> p g n", n=1024)
                nc.sync.dma_start(
                    out=w1h, in_=_ws(wfull, "w1h", "(kt p n) -> p kt n", p=128, n=1024)
                )
                wo = ktf[:, ds(8192, 4096)].rearrange("p (g n) -> p g n", n=1024)
                nc.sync.dma_start(
                    out=wo, in_=_ws(wfull, "wo", "(kt p n) -> p kt n", p=128, n=1024)
                )
                w2 = ktf[:, ds(12288, 64)].rearrange("p (g h j) -> p g h j", h=2, j=4)
                nc.sync.dma_start(
                    out=w2,
                    in_=_ws(wfull, "w2", "(kt p h j) -> p kt h j", p=128, h=2, j=4),
                )
                on = ktf[:, ds(12352, 2048)].rearrange("p (a n) -> p a n", n=512)

                for tt in range(TTS):
                    tsl = ds(tt * 512, 512)
                    hsl2 = gh.tile([128, 8, 512], BF16, tag="hsl2")
                    nc.gpsimd.dma_start(out=hsl2, in_=hT_ap[:, :, tsl])
                    for h in range(2):
                        h1 = gs.tile([128, 8, 512], BF16, tag="h1")
                        for f1 in range(8):
                            hp = psG.tile([128, 512], F32, tag="psG")
                            for g in range(8):
                                nc.tensor.matmul(
                                    hp,
                                    w1h[:, g, ds(f1 * 128, 128)],
                                    hsl2[:, g, :],
                                    start=(g == 0),
                                    stop=False,
                                )
                            nc.tensor.matmul(
                                hp,
                                w1s[ds(h * 32, 16), ds(f1 * 128, 128)],
                                stats[ds(h * 32, 16), tt, :],
                                start=False,
                                stop=True,
                            )
                            nc.scalar.activation(
                                out=h1[:, f1, :], in_=hp,
                                func=AFT.Gelu_apprx_tanh, bias=b1t[:, ds(f1, 1)],
                            )
                        lg = psL.tile([4, 512], F32, tag="psL")
                        for g in range(8):
                            nc.tensor.matmul(
                                lg, w2[:, g, h, :], h1[:, g, :],
                                start=(g == 0), stop=(g == 7),
                            )
                        ez = gr.tile([4, 512], F32, tag="r4")
                        nc.scalar.activation(
                            out=ez, in_=lg, func=AFT.Exp, bias=b2t[:, ds(h, 1)]
                        )
                        sps = psL.tile([4, 512], F32, tag="psL")
                        nc.tensor.matmul(
                            sps[0:1, :], ones_f[0:4, 0:1], ez, start=True, stop=True
                        )
                        srec = gr.tile([1, 512], F32, tag="r1")
                        nc.vector.reciprocal(srec, sps[0:1, :])
                        nc.vector.tensor_scalar_mul(srec, srec, 1.0 - 4.0 * EPS_FLOOR)
                        srb = psL.tile([4, 512], F32, tag="psL")
                        nc.tensor.matmul(
                            srb, ones_f[0:1, 0:4], srec, start=True, stop=True
                        )
                        wgt = gr.tile([4, 512], F32, tag="r4")
                        nc.vector.tensor_mul(wgt, ez, srb)
                        nc.vector.tensor_scalar_add(wgt, wgt, EPS_FLOOR)
                        o_acc = gs.tile([128, 2, 512], F32, tag="oacc")
                        for j in range(4):
                            rsp = psL.tile([4, 512], F32, tag="psL")
                            nc.tensor.matmul(
                                rsp[0:1, :], id_f32[0:4, ds(j, 1)], wgt,
                                start=True, stop=True,
                            )
                            rrow = gr.tile([1, 512], F32, tag="r1")
                            nc.vector.tensor_copy(out=rrow, in_=rsp[0:1, :])
                            wb_ = psG.tile([128, 512], F32, tag="psG")
                            nc.tensor.matmul(
                                wb_, ones_f[0:1, :], rrow, start=True, stop=True
                            )
                            for f2 in range(2):
                                if j == 0:
                                    scr = gs.tile([128, 512], F32, tag="scr")
                                    emit_fir_s(scr, 2 * h + f2, tt * 512, 512)
                                    nc.vector.tensor_mul(o_acc[:, f2, :], scr, wb_)
                                else:
                                    if j == 1:
                                        sl = flt[:, 2 * h + f2, ds(Q0 + tt * 512, 512)]
                                    elif j == 2:
                                        sl = od[:, 2 * h + f2, tsl]
                                    else:
                                        sl = vt[:, 2 * h + f2, ds(64 + tt * 512, 512)]
                                    scr = gs.tile([128, 512], F32, tag="scr")
                                    nc.vector.tensor_mul(scr, sl, wb_)
                                    nc.vector.tensor_add(
                                        o_acc[:, f2, :], o_acc[:, f2, :], scr
                                    )
                        # rms over this head's 256 features
                        rps = psL.tile([4, 512], F32, tag="psL")
                        for f2 in range(2):
                            scr = gs.tile([128, 512], F32, tag="scr")
                            nc.scalar.square(scr, o_acc[:, f2, :])
                            nc.tensor.matmul(
                                rps[0:1, :], ones_f[:, 0:1], scr,
                                start=(f2 == 0), stop=(f2 == 1),
                            )
                        rr2 = gr.tile([1, 512], F32, tag="r1")
                        nc.scalar.activation(
                            out=rr2, in_=rps[0:1, :], func=AFT.Sqrt,
                            bias=epsc[0:1, 1:2], scale=1.0 / 256.0,
                        )
                        nc.vector.reciprocal(rr2, rr2)
                        rb2 = psG.tile([128, 512], F32, tag="psG")
                        nc.tensor.matmul(rb2, ones_f[0:1, :], rr2, start=True, stop=True)
                        for f2 in range(2):
                            nc.vector.tensor_mul(
                                on[:, 2 * h + f2, :], o_acc[:, f2, :], rb2
                            )
                    # Wo partial: two halves of 4 of-tiles each
                    for half in range(2):
                        obuf = gs.tile([128, 4, 512], BF16, tag="obuf")
                        for o4 in range(4):
                            of = half * 4 + o4
                            wp = psW.tile([128, 512], F32, tag="psW")
                            for g in range(4):
                                nc.tensor.matmul(
                                    wp,
                                    wo[:, g, ds(of * 128, 128)],
                                    on[:, g, :],
                                    start=(g == 0),
                                    stop=(g == 3),
                                )
                            nc.vector.tensor_copy(out=obuf[:, o4, :], in_=wp)
                        nc.gpsimd.dma_start(
                            out=out_ap[:, ds(half * 4, 4), tsl], in_=obuf
                        )
            osc = nc.dram_tensor("osc", [512 * L], BF16, kind="Internal")
            nc.gpsimd.collective_compute(
                kind="ReduceScatter",
                op=ALU.add,
                replica_groups=PAIRS,
                ins=[opart[:]],
                outs=[osc[:]],
            )
            nc.gpsimd.dma_start(out=out_d[:, :], in_=osc[:])
    nc.compile()
    return nc


# ---------------- host side ----------------

_NC = None
_RUN = None


def _make_runner():
    global _RUN
    import jax
    from jax.sharding import Mesh, PartitionSpec
    from jax.experimental.shard_map import shard_map
    from concourse.bass2jax import (
        _bass_exec_p,
        install_neuronx_cc_hook,
        partition_id_tensor,
    )

    install_neuronx_cc_hook()
    nc = _NC
    n_cores = 8
    partition_name = nc.partition_id_tensor.name if nc.partition_id_tensor else None
    in_names, out_names, out_avals, zero_shapes = [], [], [], []
    for alloc in nc.m.functions[0].allocations:
        if not isinstance(alloc, mybir.MemoryLocationSet):
            continue
        name = alloc.memorylocations[0].name
        if alloc.kind == "ExternalInput":
            if name != partition_name:
                in_names.append(name)
        elif alloc.kind == "ExternalOutput":
            shape = tuple(alloc.tensor_shape)
            dtype = mybir.dt.np(alloc.dtype)
            out_names.append(name)
            out_avals.append(jax.core.ShapedArray(shape, dtype))
            zero_shapes.append((shape, dtype))
    n_params = len(in_names)
    all_names = list(in_names) + list(out_names)
    if partition_name is not None:
        all_names.append(partition_name)
    donate = tuple(range(n_params, n_params + len(out_names)))

    def _body(*args):
        operands = list(args)
        if partition_name is not None:
            operands.append(partition_id_tensor())
        outs = _bass_exec_p.bind(
            *operands,
            out_avals=tuple(out_avals),
            in_names=tuple(all_names),
            out_names=tuple(out_names),
            lowering_input_output_aliases=(),
            sim_require_finite=True,
            sim_require_nnan=True,
            nc=nc,
        )
        return tuple(outs)

    devices = jax.devices()[:n_cores]
    mesh = Mesh(np.asarray(devices), ("core",))
    nin = n_params + len(out_names)
    sharded = jax.jit(
        shard_map(
            _body,
            mesh=mesh,
            in_specs=(PartitionSpec("core"),) * nin,
            out_specs=(PartitionSpec("core"),) * len(out_names),
            check_rep=False,
        ),
        donate_argnums=donate,
        keep_unused=True,
    )

    state = {"donate": None}

    def run(per_core_inputs):
        concat_in = [
            np.concatenate([per_core_inputs[c][nm] for c in range(n_cores)], axis=0)
            for nm in in_names
        ]
        if state["donate"] is not None:
            zeros = state["donate"]
        else:
            zeros = [
                np.zeros((n_cores * s[0],) + tuple(s[1:]), dt) for s, dt in zero_shapes
            ]
        outs = sharded(*concat_in, *zeros)
        # keep device-resident outputs to donate as (ignored) out buffers next call
        state["donate"] = list(outs)
        return [
            {
                nm: np.asarray(outs[i]).reshape((n_cores,) + tuple(out_avals[i].shape))[c]
                for i, nm in enumerate(out_names)
            }
            for c in range(n_cores)
        ]

    _RUN = run
    return run


def _pack_weights_hg(hg, w):
    """Full bf16 weight blob for one head-pair (SEC_W layout)."""
    h0 = 2 * hg
    rows = slice(hg * 512, (hg + 1) * 512)
    blob = np.empty(NW, BF)

    def put(name, arr):
        o, sz = SEC_W[name]
        a = np.ascontiguousarray(arr, dtype=np.float32)
        assert a.size == sz, (name, a.shape, sz)
        blob[o : o + sz] = a.astype(BF).reshape(-1)

    put("wq", w["Wq"][rows].T)
    put("wk", w["Wk"][rows].T)
    put("wv", w["Wv"][rows].T)
    put("wb", w["Wb"][h0 : h0 + 2].T)
    put("w1h", w["W1h"].T)
    put("w1s", w["w1s_dup"])
    put("w2", w["w2_scaled"][:, h0 : h0 + 2, :])
    put("wo", w["Wof"][:, rows].T)

    f32v = np.zeros(NF, np.float32)

    def putf(name, arr):
        o, sz = SEC_F[name]
        a = np.ascontiguousarray(arr, dtype=np.float32)
        assert a.size == sz, (name, a.shape, sz)
        f32v[o : o + sz] = a.reshape(-1)

    cwp = np.empty((128, 4, 3, 4), np.float32)
    for pj, key in enumerate(("conv_q_w", "conv_k_w", "conv_v_w")):
        cc = np.asarray(w[key], np.float32)[rows]  # (512, 4)
        cwp[:, :, pj, :] = cc.reshape(4, 128, 4).transpose(1, 0, 2)
    putf("cw", cwp)
    fir = np.empty((128, 4, FIRS + FIRL), np.float32)
    fir[:, :, :FIRS] = w["fs"][rows].reshape(4, 128, FIRS).transpose(1, 0, 2)
    fir[:, :, FIRS:] = w["fl"][rows].reshape(4, 128, FIRL).transpose(1, 0, 2)
    putf("fir", fir)
    putf("msk", w["msk"])
    putf("b1", np.asarray(w["gate_b1"], np.float32).reshape(8, 128).T)
    putf("b2", w["b2_scaled"][:, h0 : h0 + 2])
    return blob, f32v


def _pack_all(hidden, w):
    wblobs = [_pack_weights_hg(hg, w) for hg in range(2)]
    hb = hidden.astype(BF)  # bf16 first, then cheap bf16 transposes
    per_core = []
    for c in range(8):
        b, hg = c // 2, c % 2
        half = hb[b][:, hg * 512 : (hg + 1) * 512].T  # (512g, 4096t)
        qtr = wblobs[hg][0][b * NWQ : (b + 1) * NWQ]
        bfb = np.concatenate(
            [np.ascontiguousarray(half).reshape(-1), qtr]
        )
        per_core.append({"bfb": bfb, "f32b": wblobs[hg][1]})
    return per_core


def _prep_shared(inputs):
    w = {}
    for k in ("Wq", "Wk", "Wv", "Wb", "conv_q_w", "conv_k_w", "conv_v_w", "gate_b1"):
        w[k] = np.asarray(inputs[k], np.float32)
    W1 = np.asarray(inputs["gate_W1"], np.float32)
    w["W1h"] = W1[:, :HS]
    W1s = W1[:, HS:].copy()
    for blk in range(4):
        W1s[:, blk * 4 + 0] *= 1.0 / 256.0
        W1s[:, blk * 4 + 2] *= 1.0 / 256.0
    w1sT = W1s.T  # (16, 1024)
    dup = np.zeros((64, 1024), np.float32)
    dup[0:16] = w1sT
    dup[32:48] = w1sT
    w["w1s_dup"] = dup
    temp = np.exp(np.asarray(inputs["gate_log_temp"], np.float32))  # (NH,)
    W2 = np.asarray(inputs["gate_W2"], np.float32)  # (4, 1024)
    w2s = np.empty((HS, NH, 4), np.float32)
    for h in range(NH):
        w2s[:, h, :] = (W2 / temp[h]).T
    w["w2_scaled"] = w2s
    b2 = np.asarray(inputs["gate_b2"], np.float32)
    bias_val = np.asarray(inputs["gate_copy_bias"], np.float32) * DECAY
    b2s = np.empty((4, NH), np.float32)
    for h in range(NH):
        b2s[:, h] = b2 / temp[h]
        b2s[3, h] += bias_val[h] / temp[h]
    w["b2_scaled"] = b2s
    onw = np.asarray(inputs["o_norm_w"], np.float32)
    w["Wof"] = np.asarray(inputs["Wo"], np.float32) * np.tile(onw, NH)[None, :]
    w["fs"] = np.asarray(inputs["fir_short_filt"], np.float32).reshape(NH * DV, FIRS)
    w["fl"] = np.asarray(inputs["fir_long_filt"], np.float32).reshape(NH * DV, FIRL)
    msk = np.zeros((128, 256), np.float32)
    ii = np.arange(128)
    msk[:, :128][ii[:, None] > ii[None, :]] = -1.0  # strict lower
    msk[:, 128:][ii[:, None] <= ii[None, :]] = 1.0  # upper incl diag (attnT)
    w["msk"] = msk
    return w


def _ensure_ready():
    global _NC, _RUN
    if _RUN is None:
        _NC = _build_nc()
        _make_runner()


def _warmup():
    _ensure_ready()
    per_core = [
        {"bfb": np.zeros(NBF, BF), "f32b": np.zeros(NF, np.float32)} for _ in range(8)
    ]
    _RUN(per_core)


def kernel(**inputs):
    _ensure_ready()
    hidden = np.asarray(inputs["hidden_states"], np.float32)
    w = _prep_shared(inputs)
    per_core = _pack_all(hidden, w)
    res = _RUN(per_core)
    out = np.empty((B, L, HS), np.float32)
    for b in range(B):
        full = np.concatenate(
            [np.asarray(res[2 * b]["out"]), np.asarray(res[2 * b + 1]["out"])], axis=0
        )
        out[b] = full.astype(np.float32).T
    return out


import os as _os

if not _os.environ.get("KERNEL_NO_WARMUP"):
    try:
        _warmup()
    except Exception as _e:  # pragma: no cover
        import traceback

        traceback.print_exc()
